# revision 1
# baseline (speedup 1.0000x reference)
"""BiGCN (two-branch GCN + root-extend + scatter-mean + MLP) on 8 trn2 NeuronCores.

Sharding: nodes/edges are sharded by destination across 8 cores using
graph-aligned windows (so scatter-mean pooling stays core-local); the small
weight matrices are replicated. Two SPMD launches (layer-1, then
layer-2+pool+MLP) with host reassembly of layer-1 activations in between.

Per conv layer on device: build the full normalized feature table
ht = dinv * (act @ W) in DRAM (node-major), then per 128-dst-node tile:
indirect-DMA gather of ht[src] messages, one-hot A01 = (dstlocal == iota)
built on the vector engine, PE matmul segment-sum into PSUM (+ self-loop via
identity matmul, + bias), scale by dinv on evacuation.
"""
import numpy as np

import concourse.bacc as bacc
import concourse.mybir as mybir
import concourse.tile as tile
from concourse.bass_utils import run_bass_kernel_spmd

P = 128
N_CORES = 8
F32 = mybir.dt.float32
I32 = mybir.dt.int32

# table/message storage dtype (PSUM accumulation is always fp32)
TBL_DT = mybir.dt.float32
TBL_NP = np.float32


# ----------------------------------------------------------------------------
# host-side preprocessing (index manipulation only)
# ----------------------------------------------------------------------------

def _ceil(a, b):
    return -(-a // b)


def _shard_meta(batch, B, N):
    """Graph-aligned per-core node windows, padded to a uniform 128-aligned
    size. Core c owns graphs [g0[c], g0[c+1]); it computes a window of NLOC
    nodes starting at its first owned node (covering all owned graphs plus a
    partial tail that is discarded)."""
    node_start = np.searchsorted(batch, np.arange(B + 1))
    g0 = [int(_ceil(B * c, N_CORES)) for c in range(N_CORES + 1)]
    spans = [int(node_start[g0[c + 1]] - node_start[g0[c]]) for c in range(N_CORES)]
    NLOC = _ceil(max(spans), P) * P
    T = NLOC // P
    n0 = [int(node_start[g0[c]]) for c in range(N_CORES)]
    gcounts = []
    for c in range(N_CORES):
        hi = min(n0[c] + NLOC, N)
        glast = int(batch[hi - 1]) if hi > n0[c] else g0[c]
        gcounts.append(glast - g0[c] + 1)
    G_LOC = max(gcounts)
    assert G_LOC <= P, f"G_LOC={G_LOC} exceeds 128 partitions"
    return {"node_start": node_start, "g0": g0, "n0": n0, "NLOC": NLOC,
            "T": T, "G_LOC": G_LOC}


CHK = 32768  # dma_gather table-chunk rows (int16 index range)


def _edges_for_core(src, dst, n0, NLOC, N, T, NQ):
    """Edges with dst in this core's window PLUS one self-edge per real
    window node (the GCN self-loop has exactly the edge normalization
    dinv_d*dinv_d, so it is just an extra (d, d) edge). Sorted by
    (dst tile, src); per-(tile, src-chunk) counts."""
    lo, hi = n0, min(n0 + NLOC, N)
    m = (dst >= lo) & (dst < hi)
    es = src[m].astype(np.int64)
    ed = (dst[m] - lo).astype(np.int64)
    sl = np.arange(lo, hi, dtype=np.int64)
    es = np.concatenate([es, sl])
    ed = np.concatenate([ed, sl - lo])
    tl = ed >> 7
    order = np.lexsort((es, tl))
    es, ed, tl = es[order], ed[order], tl[order]
    q = es >> 15
    cnt_tq = np.bincount(tl * NQ + q, minlength=T * NQ).reshape(T, NQ)
    return es, ed, tl, q, cnt_tq


def _pack_edges(branch_cores, T, NQ):
    """Union-max per-(tile, chunk) block counts sb[t][q]; per-core padded
    arrays: IDX16 [128, Mbar*8] int16 (dma_gather wrapped layout, idx
    relative to chunk, pad=0) and DSTL [128, Mbar] f32 (pad=-1). Flat edge
    slot j of segment (t,q) at block boff[t][q]+j//128, partition j%128 —
    exactly dma_gather's output layout."""
    sb = np.stack([(c["cnt_tq"] + P - 1) // P for c in branch_cores]).max(axis=0)
    boff = np.concatenate([[0], np.cumsum(sb.ravel())]).reshape(-1)[:-1].reshape(T, NQ)
    mb = sb.sum(axis=1)
    off = np.concatenate([[0], np.cumsum(mb)])
    Mbar = max(1, int(off[-1]))
    out = []
    for c in branch_cores:
        F = np.zeros(Mbar * P, np.int16)
        DSTL = np.full((P, Mbar), -1.0, np.float32)
        es, ed, tl, q, cnt_tq = (c["es"], c["ed"], c["tl"], c["q"], c["cnt_tq"])
        if len(ed):
            segid = tl * NQ + q
            starts = np.concatenate([[0], np.cumsum(cnt_tq.ravel())])
            within = np.arange(len(ed)) - starts[segid]
            flat = boff.ravel()[segid] * P + within
            F[flat] = (es & (CHK - 1)).astype(np.int16)
            DSTL[flat & 127, flat >> 7] = (ed - (tl << 7)).astype(np.float32)
        IDX16 = np.ascontiguousarray(np.tile(F.reshape(-1, 16).T, (8, 1)))
        out.append({"IDX16": IDX16, "DSTL": DSTL})
    return (sb.astype(int), boff.astype(int), mb.astype(int).tolist(),
            off.astype(int), Mbar, out)


def _part_major(vec, TB, fill):
    v = np.full(TB * P, fill, vec.dtype)
    v[: len(vec)] = vec
    return np.ascontiguousarray(v.reshape(TB, P).T)


def preprocess(x, x_da, edge_index, batch, rootindex):
    N = x.shape[0]
    B = rootindex.shape[0]
    x0 = np.concatenate([x, x_da], axis=1).astype(np.float32)
    assert x0.shape[1] == P
    TBL = _ceil(N, P)
    x0T = np.zeros((P, TBL * P), np.float32)
    x0T[:, :N] = x0.T
    batch = batch.astype(np.int64)
    meta = _shard_meta(batch, B, N)
    T, NLOC, G_LOC = meta["T"], meta["NLOC"], meta["G_LOC"]

    src_g = edge_index[0].astype(np.int64)
    dst_g = edge_index[1].astype(np.int64)

    NQ = _ceil(TBL * P, CHK)
    branches = {}
    for name, (s, d) in {"td": (src_g, dst_g), "bu": (dst_g, src_g)}.items():
        deg = (np.bincount(d, minlength=N) + 1.0).astype(np.float32)
        dinv = (1.0 / np.sqrt(deg)).astype(np.float32)
        cores = []
        for c in range(N_CORES):
            es, ed, tl, q, cnt_tq = _edges_for_core(
                s, d, meta["n0"][c], NLOC, N, T, NQ)
            cores.append({"es": es, "ed": ed, "tl": tl, "q": q,
                          "cnt_tq": cnt_tq})
        sb, boff, mb, off, Mbar, packed = _pack_edges(cores, T, NQ)
        bd = {"dinv": dinv, "sb": sb, "boff": boff, "mbar": mb, "off": off,
              "Mbar": Mbar, "packed": packed,
              "dinv_part": _part_major(dinv, TBL, np.float32(1.0))}
        loc_dinv = []
        for c in range(N_CORES):
            rows = meta["n0"][c] + np.arange(NLOC)
            valid = rows < N
            rr = np.minimum(rows, N - 1)
            dv = np.where(valid, dinv[rr], 1.0).astype(np.float32)
            loc_dinv.append(np.ascontiguousarray(dv.reshape(T, P).T))
        bd["dinv_loc"] = loc_dinv
        branches[name] = bd

    GL, cinv = [], []
    for c in range(N_CORES):
        rows = meta["n0"][c] + np.arange(NLOC)
        valid = rows < N
        rr = np.minimum(rows, N - 1)
        gl = np.where(valid, batch[rr] - meta["g0"][c], -1).astype(np.float32)
        gl = np.where(gl < G_LOC, gl, -1.0).astype(np.float32)
        GL.append(np.ascontiguousarray(gl.reshape(T, P).T))
        cnts = np.ones(P, np.float32)
        ns = meta["node_start"]
        for j in range(G_LOC):
            g = meta["g0"][c] + j
            if g < B:
                cc = float(ns[g + 1] - ns[g])
                cnts[j] = cc if cc > 0 else 1.0
        cinv.append((1.0 / cnts).reshape(P, 1).astype(np.float32))

    # node -> graph ids, dma_gather wrapped int16 layout (B < 32768)
    ngf = np.zeros(TBL * P, np.int16)
    ngf[:N] = batch.astype(np.int16)
    NG16 = np.ascontiguousarray(np.tile(ngf.reshape(-1, 16).T, (8, 1)))

    B_PAD = _ceil(B, P) * P
    rootx0T = np.zeros((P, B_PAD), np.float32)
    rootx0T[:, :B] = x0[rootindex.astype(np.int64)].T

    iota = np.broadcast_to(np.arange(P, dtype=np.float32), (P, P)).copy()
    ident = np.eye(P, dtype=np.float32)

    return {"N": N, "B": B, "TBL": TBL, "B_PAD": B_PAD, "NQ": NQ,
            "meta": meta, "x0": x0, "x0T": x0T, "branches": branches,
            "GL": GL, "cinv": cinv, "NG16": NG16, "rootx0T": rootx0T,
            "iota": iota, "ident": ident}


# ----------------------------------------------------------------------------
# device program builders
# ----------------------------------------------------------------------------

import os
N_QUEUES = int(os.environ.get("K_QUEUES", "4"))
K_SP = os.environ.get("K_SP", "0") == "1"   # single_packet for gathers
_qctr = [0]


def _next_q():
    q = _qctr[0] % N_QUEUES
    _qctr[0] += 1
    return q


def _new_nc():
    return bacc.Bacc("TRN2", target_bir_lowering=False, debug=False,
                     num_devices=N_CORES, num_swdge_queues=N_QUEUES)


def _load(nc, pool, dram_ap, shape, dtype, tag, bufs=1):
    t = pool.tile(list(shape), dtype, tag=tag, bufs=bufs)
    nc.sync.dma_start(out=t[:], in_=dram_ap)
    return t


def _bias_tile(nc, pool, psum, ones_sb, b_sb, tag):
    """[128,128] SBUF tile holding the bias row broadcast to every partition."""
    psb = psum.tile([P, P], F32, tag="ps")
    nc.tensor.matmul(psb[:], lhsT=ones_sb[0:1, :], rhs=b_sb[0:1, :],
                     start=True, stop=True)
    bt = pool.tile([P, P], F32, tag=tag, bufs=1)
    nc.vector.tensor_copy(out=bt[:], in_=psb[:])
    return bt


def _aggregate(nc, pool, psum, table, IDX_sb, DSTL_sb, bmeta, TROWS,
               iota_sb, T, consume):
    """Per-tile segment-sum: PSUM_t = sum_e A01 . msg (self-loops are real
    edges); then consume(t, ps) finishes (scale/bias/relu/pool/write).
    Messages are gathered per (tile, 32k-row table chunk) with dma_gather."""
    sb, boff, mbar, off = bmeta["sb"], bmeta["boff"], bmeta["mbar"], bmeta["off"]
    NQ = sb.shape[1]
    mbmax = max(1, max(mbar))
    for t in range(T):
        mb = mbar[t]
        if mb == 0:
            continue  # window tail beyond N: no nodes, nothing to write
        msg = pool.tile([P, mbmax * P], TBL_DT, tag="msg", bufs=3)
        col = 0
        for q in range(NQ):
            nb = int(sb[t][q])
            if nb == 0:
                continue
            base = q * CHK
            rows = min(CHK, TROWS - base)
            nc.gpsimd.dma_gather(
                out_ap=msg[:, col * P: (col + nb) * P]
                .rearrange("p (b f) -> p b f", f=P),
                in_ap=table[base: base + rows, :],
                idxs_ap=IDX_sb[:, boff[t][q] * 8: (boff[t][q] + nb) * 8],
                num_idxs=nb * P, num_idxs_reg=nb * P, elem_size=P)
            col += nb
        a01 = pool.tile([P, mbmax * P], TBL_DT, tag="a01", bufs=3)
        nc.vector.tensor_tensor(
            out=a01[:, : mb * P].rearrange("p (k f) -> p k f", f=P),
            in0=DSTL_sb[:, off[t]: off[t] + mb].to_broadcast([P, mb, P]),
            in1=iota_sb[:].unsqueeze(1).broadcast_to([P, mb, P]),
            op=mybir.AluOpType.is_equal,
        )
        ps = psum.tile([P, P], F32, tag="ps")
        for k in range(mb):
            nc.tensor.matmul(ps[:], lhsT=a01[:, k * P: (k + 1) * P],
                             rhs=msg[:, k * P: (k + 1) * P],
                             start=(k == 0), stop=(k == mb - 1))
        consume(t, ps)


def build_l1(pp, reps=1):
    TBL, T = pp["TBL"], pp["meta"]["T"]
    br = pp["branches"]
    nc = _new_nc()
    x0T = nc.dram_tensor("x0T", [P, TBL * P], F32, kind="ExternalInput")
    iota = nc.dram_tensor("iota", [P, P], F32, kind="ExternalInput")
    ones_row = nc.dram_tensor("ones_row", [1, P], F32, kind="ExternalInput")
    ins = {}
    for b in ("td", "bu"):
        M = br[b]["Mbar"]
        ins[b] = {
            "w1": nc.dram_tensor(f"w1{b}", [P, P], F32, kind="ExternalInput"),
            "b1": nc.dram_tensor(f"b1{b}", [1, P], F32, kind="ExternalInput"),
            "dinv_part": nc.dram_tensor(f"dinvp{b}", [P, TBL], F32, kind="ExternalInput"),
            "dinv_loc": nc.dram_tensor(f"dinvl{b}", [P, T], F32, kind="ExternalInput"),
            "IDX": nc.dram_tensor(f"IDX{b}", [P, M * 8], mybir.dt.int16,
                                  kind="ExternalInput"),
            "DSTL": nc.dram_tensor(f"DSTL{b}", [P, M], F32, kind="ExternalInput"),
            "table": nc.dram_tensor(f"table{b}", [TBL * P, P], TBL_DT, kind="Internal"),
            "x2": nc.dram_tensor(f"x2{b}", [T * P, P], F32, kind="ExternalOutput"),
        }

    with tile.TileContext(nc) as tc:
        with (
            tc.tile_pool(name="sbuf", bufs=2) as pool,
            tc.tile_pool(name="cst", bufs=1) as cst,
            tc.tile_pool(name="psum", bufs=4, space="PSUM") as psum,
        ):
            iota_sb = _load(nc, cst, iota[:], (P, P), F32, "iota")
            ones_sb = _load(nc, cst, ones_row[:], (1, P), F32, "ones")
            w1_sb = {b: _load(nc, cst, ins[b]["w1"][:], (P, P), F32, f"w1{b}")
                     for b in ("td", "bu")}
            dinvp_sb = {b: _load(nc, cst, ins[b]["dinv_part"][:], (P, TBL), F32,
                                 f"dinvp{b}") for b in ("td", "bu")}

            import contextlib
            loop_ctx = tc.For_i(0, reps, 1) if reps > 1 else contextlib.nullcontext()
            with loop_ctx:
                _build_l1_body(nc, x0T, pool, cst, psum, pp, ins, iota_sb,
                               ones_sb, w1_sb, dinvp_sb)
    nc.compile()
    return nc


def _build_l1_body(nc, x0T, pool, cst, psum, pp, ins, iota_sb, ones_sb,
                   w1_sb, dinvp_sb):
    TBL, T = pp["TBL"], pp["meta"]["T"]
    br = pp["branches"]
    if True:
        if True:
            # ---- tables: ht_b = dinv_b * (x0 @ W1_b), node-major in DRAM ----
            CH = 8
            for c0 in range(0, TBL, CH):
                nb = min(CH, TBL - c0)
                xt = pool.tile([P, CH * P], F32, tag="xt", bufs=2)
                nc.sync.dma_start(out=xt[:, : nb * P],
                                  in_=x0T[:, c0 * P: (c0 + nb) * P])
                st = {b: pool.tile([P, CH * P], TBL_DT, tag=f"st{b}", bufs=2,
                                   name=f"st{b}")
                      for b in ("td", "bu")}
                for j in range(nb):
                    blk = c0 + j
                    for b in ("td", "bu"):
                        psx = psum.tile([P, P], F32, tag="ps")
                        nc.tensor.matmul(psx[:], lhsT=xt[:, j * P: (j + 1) * P],
                                         rhs=w1_sb[b][:], start=True, stop=True)
                        nc.vector.tensor_scalar(
                            out=st[b][:, j * P: (j + 1) * P], in0=psx[:],
                            scalar1=dinvp_sb[b][:, blk: blk + 1], scalar2=None,
                            op0=mybir.AluOpType.mult)
                for b in ("td", "bu"):
                    nc.sync.dma_start(
                        out=ins[b]["table"][c0 * P: (c0 + nb) * P, :]
                        .rearrange("(j p) f -> p j f", p=P),
                        in_=st[b][:, : nb * P].rearrange("p (j f) -> p j f", f=P))

            # ---- aggregation per branch ----
            for b in ("td", "bu"):
                ib = ins[b]
                M = br[b]["Mbar"]
                IDX_sb = _load(nc, pool, ib["IDX"][:], (P, M * 8),
                               mybir.dt.int16, "idx")
                DSTL_sb = _load(nc, pool, ib["DSTL"][:], (P, M), F32, "dstl")
                dinvl_sb = _load(nc, cst, ib["dinv_loc"][:], (P, T), F32, f"dinvl{b}")
                b1_sb = _load(nc, cst, ib["b1"][:], (1, P), F32, f"b1{b}")
                btile = _bias_tile(nc, pool, psum, ones_sb, b1_sb, f"btile{b}")

                def consume(t, ps, ib=ib, dinvl_sb=dinvl_sb, btile=btile):
                    xo = pool.tile([P, P], F32, tag="xo", bufs=3)
                    nc.vector.tensor_scalar(
                        out=xo[:], in0=ps[:], scalar1=dinvl_sb[:, t: t + 1],
                        scalar2=None, op0=mybir.AluOpType.mult)
                    nc.vector.tensor_add(out=xo[:], in0=xo[:], in1=btile[:])
                    nc.sync.dma_start(out=ib["x2"][t * P: (t + 1) * P, :], in_=xo[:])

                _aggregate(nc, pool, psum, ib["table"], IDX_sb, DSTL_sb,
                           br[b], TBL * P, iota_sb, T, consume)


def build_l2(pp, reps=1):
    TBL, T = pp["TBL"], pp["meta"]["T"]
    G_LOC, B_PAD = pp["meta"]["G_LOC"], pp["B_PAD"]
    br = pp["branches"]
    nc = _new_nc()
    iota = nc.dram_tensor("iota", [P, P], F32, kind="ExternalInput")
    ident = nc.dram_tensor("ident", [P, P], TBL_DT, kind="ExternalInput")
    NG = nc.dram_tensor("NG", [P, TBL * 8], mybir.dt.int16, kind="ExternalInput")
    GL = nc.dram_tensor("GL", [P, T], F32, kind="ExternalInput")
    cinv = nc.dram_tensor("cinv", [P, 1], F32, kind="ExternalInput")
    rootx0T = nc.dram_tensor("rootx0T", [P, B_PAD], F32, kind="ExternalInput")
    ones_row = nc.dram_tensor("ones_row", [1, P], F32, kind="ExternalInput")
    mlp_w1 = nc.dram_tensor("mlp_w1", [4 * P, 2 * P], F32, kind="ExternalInput")
    mlp_b1 = nc.dram_tensor("mlp_b1", [1, 2 * P], F32, kind="ExternalInput")
    mlp_w2 = nc.dram_tensor("mlp_w2", [2 * P, 2], F32, kind="ExternalInput")
    mlp_b2 = nc.dram_tensor("mlp_b2", [1, 2], F32, kind="ExternalInput")
    out = nc.dram_tensor("out", [P, 2], F32, kind="ExternalOutput")
    ins = {}
    for b in ("td", "bu"):
        M = br[b]["Mbar"]
        ins[b] = {
            "x2T": nc.dram_tensor(f"x2T{b}", [P, TBL * P], F32, kind="ExternalInput"),
            "w2a": nc.dram_tensor(f"w2a{b}", [P, P], F32, kind="ExternalInput"),
            "w2b": nc.dram_tensor(f"w2b{b}", [P, P], F32, kind="ExternalInput"),
            "b2": nc.dram_tensor(f"b2{b}", [1, P], F32, kind="ExternalInput"),
            "dinv_part": nc.dram_tensor(f"dinvp{b}", [P, TBL], F32, kind="ExternalInput"),
            "dinv_loc": nc.dram_tensor(f"dinvl{b}", [P, T], F32, kind="ExternalInput"),
            "IDX": nc.dram_tensor(f"IDX{b}", [P, M * 8], mybir.dt.int16,
                                  kind="ExternalInput"),
            "DSTL": nc.dram_tensor(f"DSTL{b}", [P, M], F32, kind="ExternalInput"),
            "rootx2T": nc.dram_tensor(f"rx2T{b}", [P, P], F32, kind="ExternalInput"),
            "rtab": nc.dram_tensor(f"rtab{b}", [B_PAD, P], F32, kind="Internal"),
            "table": nc.dram_tensor(f"table{b}", [TBL * P, P], TBL_DT, kind="Internal"),
        }

    with tile.TileContext(nc) as tc:
        with (
            tc.tile_pool(name="sbuf", bufs=2) as pool,
            tc.tile_pool(name="cst", bufs=1) as cst,
            tc.tile_pool(name="psum", bufs=4, space="PSUM") as psum,
            tc.tile_pool(name="pps", bufs=1, space="PSUM") as pool_ps,
        ):
            iota_sb = _load(nc, cst, iota[:], (P, P), F32, "iota")
            ident_sb = _load(nc, cst, ident[:], (P, P), TBL_DT, "ident")
            ones_sb = _load(nc, cst, ones_row[:], (1, P), F32, "ones")
            NG_sb = _load(nc, cst, NG[:], (P, TBL * 8), mybir.dt.int16, "NG")
            GL_sb = _load(nc, cst, GL[:], (P, T), F32, "GL")
            cinv_sb = _load(nc, cst, cinv[:], (P, 1), F32, "cinv")
            w2a_sb = {b: _load(nc, cst, ins[b]["w2a"][:], (P, P), F32, f"w2a{b}")
                      for b in ("td", "bu")}
            w2b_sb = {b: _load(nc, cst, ins[b]["w2b"][:], (P, P), F32, f"w2b{b}")
                      for b in ("td", "bu")}
            dinvp_sb = {b: _load(nc, cst, ins[b]["dinv_part"][:], (P, TBL), F32,
                                 f"dinvp{b}") for b in ("td", "bu")}

            import contextlib
            loop_ctx = tc.For_i(0, reps, 1) if reps > 1 else contextlib.nullcontext()
            with loop_ctx:
                _build_l2_body(
                    nc, pool, cst, psum, pool_ps, pp, ins, rootx0T, mlp_w1,
                    mlp_b1, mlp_w2, mlp_b2, out, iota_sb, ident_sb, ones_sb,
                    NG_sb, GL_sb, cinv_sb, w2a_sb, w2b_sb, dinvp_sb)
    nc.compile()
    return nc


def _build_l2_body(nc, pool, cst, psum, pool_ps, pp, ins, rootx0T, mlp_w1,
                   mlp_b1, mlp_w2, mlp_b2, out, iota_sb, ident_sb, ones_sb,
                   NG_sb, GL_sb, cinv_sb, w2a_sb, w2b_sb, dinvp_sb):
    TBL, T = pp["TBL"], pp["meta"]["T"]
    G_LOC, B_PAD = pp["meta"]["G_LOC"], pp["B_PAD"]
    br = pp["branches"]
    if True:
        if True:
            # ---- R tables: R_b = relu(x0[roots]) @ W2b_b ----
            for j in range(B_PAD // P):
                rx = pool.tile([P, P], F32, tag="rx", bufs=2)
                nc.sync.dma_start(out=rx[:], in_=rootx0T[:, j * P: (j + 1) * P])
                rr = pool.tile([P, P], F32, tag="rr", bufs=2)
                nc.scalar.activation(out=rr[:], in_=rx[:],
                                     func=mybir.ActivationFunctionType.Relu)
                for b in ("td", "bu"):
                    psr = psum.tile([P, P], F32, tag="ps")
                    nc.tensor.matmul(psr[:], lhsT=rr[:], rhs=w2b_sb[b][:],
                                     start=True, stop=True)
                    ro = pool.tile([P, P], F32, tag="ro", bufs=2)
                    nc.vector.tensor_copy(out=ro[:], in_=psr[:])
                    nc.sync.dma_start(out=ins[b]["rtab"][j * P: (j + 1) * P, :],
                                      in_=ro[:])

            # ---- ht2 tables: dinv_b * (relu(x2_b) @ W2a_b + R_b[batch]) ----
            CH = 8
            for b in ("td", "bu"):
                ib = ins[b]
                for c0 in range(0, TBL, CH):
                    nb = min(CH, TBL - c0)
                    xt = pool.tile([P, CH * P], F32, tag="xt", bufs=2)
                    nc.sync.dma_start(out=xt[:, : nb * P],
                                      in_=ib["x2T"][:, c0 * P: (c0 + nb) * P])
                    xr = pool.tile([P, CH * P], F32, tag="xr", bufs=2)
                    nc.scalar.activation(out=xr[:, : nb * P], in_=xt[:, : nb * P],
                                         func=mybir.ActivationFunctionType.Relu)
                    rg = pool.tile([P, CH * P], F32, tag="rg", bufs=2)
                    nc.gpsimd.dma_gather(
                        out_ap=rg[:, : nb * P].rearrange("p (b f) -> p b f", f=P),
                        in_ap=ib["rtab"][:, :],
                        idxs_ap=NG_sb[:, c0 * 8: (c0 + nb) * 8],
                        num_idxs=nb * P, num_idxs_reg=nb * P, elem_size=P,
                        queue_num=_next_q(), single_packet=K_SP)
                    st = pool.tile([P, CH * P], TBL_DT, tag="st", bufs=2)
                    for j in range(nb):
                        blk = c0 + j
                        psx = psum.tile([P, P], F32, tag="ps")
                        nc.tensor.matmul(psx[:], lhsT=xr[:, j * P: (j + 1) * P],
                                         rhs=w2a_sb[b][:], start=True, stop=False)
                        nc.tensor.matmul(psx[:], lhsT=ident_sb[:],
                                         rhs=rg[:, j * P: (j + 1) * P],
                                         start=False, stop=True)
                        nc.vector.tensor_scalar(
                            out=st[:, j * P: (j + 1) * P], in0=psx[:],
                            scalar1=dinvp_sb[b][:, blk: blk + 1], scalar2=None,
                            op0=mybir.AluOpType.mult)
                    nc.sync.dma_start(
                        out=ib["table"][c0 * P: (c0 + nb) * P, :]
                        .rearrange("(j p) f -> p j f", p=P),
                        in_=st[:, : nb * P].rearrange("p (j f) -> p j f", f=P))

            # ---- aggregation + relu + pooling per branch ----
            pooled = {}
            for b in ("td", "bu"):
                ib = ins[b]
                M = br[b]["Mbar"]
                IDX_sb = _load(nc, pool, ib["IDX"][:], (P, M * 8),
                               mybir.dt.int16, "idx")
                DSTL_sb = _load(nc, pool, ib["DSTL"][:], (P, M), F32, "dstl")
                dinvl_sb = _load(nc, cst, ib["dinv_loc"][:], (P, T), F32, f"dinvl{b}")
                b2_sb = _load(nc, cst, ib["b2"][:], (1, P), F32, f"b2{b}")
                btile = _bias_tile(nc, pool, psum, ones_sb, b2_sb, f"btile{b}")
                ps_pool = pool_ps.tile([G_LOC, P], F32, tag=f"pool{b}")
                t_last = max(t for t in range(T) if br[b]["mbar"][t] > 0)

                def consume(t, ps, dinvl_sb=dinvl_sb, btile=btile,
                            ps_pool=ps_pool, t_last=t_last):
                    hs = pool.tile([P, P], F32, tag="hs", bufs=3)
                    nc.vector.tensor_scalar(
                        out=hs[:], in0=ps[:], scalar1=dinvl_sb[:, t: t + 1],
                        scalar2=None, op0=mybir.AluOpType.mult)
                    nc.vector.tensor_add(out=hs[:], in0=hs[:], in1=btile[:])
                    h2 = pool.tile([P, P], TBL_DT, tag="h2", bufs=3)
                    nc.scalar.activation(out=h2[:], in_=hs[:],
                                         func=mybir.ActivationFunctionType.Relu)
                    oh = pool.tile([P, G_LOC], TBL_DT, tag="oh", bufs=3)
                    nc.vector.tensor_tensor(
                        out=oh[:], in0=GL_sb[:, t: t + 1].to_broadcast([P, G_LOC]),
                        in1=iota_sb[:, :G_LOC], op=mybir.AluOpType.is_equal)
                    nc.tensor.matmul(ps_pool[:], lhsT=oh[:], rhs=h2[:],
                                     start=(t == 0), stop=(t == t_last))

                _aggregate(nc, pool, psum, ib["table"], IDX_sb, DSTL_sb,
                           br[b], TBL * P, iota_sb, T, consume)

                meanS = pool.tile([G_LOC, P], F32, tag=f"mean{b}", bufs=1)
                nc.vector.tensor_scalar(
                    out=meanS[:], in0=ps_pool[:], scalar1=cinv_sb[:G_LOC, :],
                    scalar2=None, op0=mybir.AluOpType.mult)
                pst = psum.tile([P, G_LOC], F32, tag="ps")
                nc.tensor.transpose(out=pst[:], in_=meanS[:],
                                    identity=ident_sb[:G_LOC, :G_LOC])
                meanT = pool.tile([P, G_LOC], F32, tag=f"meanT{b}", bufs=1)
                nc.vector.tensor_copy(out=meanT[:], in_=pst[:])
                pooled[b] = meanT

            # ---- final MLP over this core's G_LOC graph slots ----
            rx2_sb = {b: _load(nc, cst, ins[b]["rootx2T"][:, :P], (P, P), F32,
                               f"rx2{b}") for b in ("td", "bu")}
            w1c_sb = cst.tile([P, 4 * 2 * P], F32, tag="mw1", name="mw1")
            nc.sync.dma_start(
                out=w1c_sb[:].rearrange("p (c o) -> p c o", c=4),
                in_=mlp_w1[:].rearrange("(c p) o -> p c o", p=P))
            b1m_sb = _load(nc, cst, mlp_b1[:], (1, 2 * P), F32, "mb1")
            w2c_sb = cst.tile([P, 2 * 2], F32, tag="mw2", name="mw2")
            nc.sync.dma_start(
                out=w2c_sb[:].rearrange("p (c o) -> p c o", c=2),
                in_=mlp_w2[:].rearrange("(c p) o -> p c o", p=P))
            b2m_sb = _load(nc, cst, mlp_b2[:], (1, 2), F32, "mb2")

            ps1 = psum.tile([G_LOC, 2 * P], F32, tag="ps")
            chunks = [pooled["bu"][:, :G_LOC], rx2_sb["bu"][:, :G_LOC],
                      pooled["td"][:, :G_LOC], rx2_sb["td"][:, :G_LOC]]
            for ci, lhsT in enumerate(chunks):
                nc.tensor.matmul(ps1[:], lhsT=lhsT,
                                 rhs=w1c_sb[:, ci * 2 * P: (ci + 1) * 2 * P],
                                 start=(ci == 0), stop=False)
            nc.tensor.matmul(ps1[:], lhsT=ones_sb[0:1, :G_LOC], rhs=b1m_sb[0:1, :],
                             start=False, stop=True)
            h1 = pool.tile([G_LOC, 2 * P], F32, tag="mlph", bufs=1)
            nc.scalar.activation(out=h1[:], in_=ps1[:],
                                 func=mybir.ActivationFunctionType.Relu)
            hT = []
            for ci in range(2):
                pst2 = psum.tile([P, G_LOC], F32, tag="ps")
                nc.tensor.transpose(out=pst2[:], in_=h1[:, ci * P: (ci + 1) * P],
                                    identity=ident_sb[:G_LOC, :G_LOC])
                ht_sb = pool.tile([P, G_LOC], F32, tag=f"hT{ci}", bufs=1)
                nc.vector.tensor_copy(out=ht_sb[:], in_=pst2[:])
                hT.append(ht_sb)
            ps2 = psum.tile([G_LOC, 2], F32, tag="ps")
            for ci in range(2):
                nc.tensor.matmul(ps2[:], lhsT=hT[ci][:, :G_LOC],
                                 rhs=w2c_sb[:, ci * 2: (ci + 1) * 2],
                                 start=(ci == 0), stop=False)
            nc.tensor.matmul(ps2[:], lhsT=ones_sb[0:1, :G_LOC], rhs=b2m_sb[0:1, :],
                             start=False, stop=True)
            oo = pool.tile([G_LOC, 2], F32, tag="oo", bufs=1)
            nc.vector.tensor_copy(out=oo[:], in_=ps2[:])
            nc.sync.dma_start(out=out[:G_LOC, :], in_=oo[:])


# ----------------------------------------------------------------------------
# in_map assembly + kernel entry
# ----------------------------------------------------------------------------

def l1_in_maps(pp, w):
    br = pp["branches"]
    ones = np.ones((1, P), np.float32)
    maps = []
    for c in range(N_CORES):
        m = {"x0T": pp["x0T"], "iota": pp["iota"], "ones_row": ones}
        for b in ("td", "bu"):
            bb = br[b]
            m[f"w1{b}"] = np.ascontiguousarray(w[f"{b}_w1"].astype(np.float32))
            m[f"b1{b}"] = w[f"{b}_b1"].astype(np.float32).reshape(1, P)
            m[f"dinvp{b}"] = bb["dinv_part"]
            m[f"dinvl{b}"] = bb["dinv_loc"][c]
            m[f"IDX{b}"] = bb["packed"][c]["IDX16"]
            m[f"DSTL{b}"] = bb["packed"][c]["DSTL"]
        maps.append(m)
    return maps


def l2_in_maps(pp, w, x2T, rootx2T):
    br = pp["branches"]
    ones = np.ones((1, P), np.float32)
    maps = []
    for c in range(N_CORES):
        m = {"iota": pp["iota"], "ident": pp["ident"].astype(TBL_NP),
             "NG": pp["NG16"], "GL": pp["GL"][c],
             "cinv": pp["cinv"][c], "rootx0T": pp["rootx0T"], "ones_row": ones,
             "mlp_w1": w["mlp_w1"].astype(np.float32),
             "mlp_b1": w["mlp_b1"].astype(np.float32).reshape(1, -1),
             "mlp_w2": w["mlp_w2"].astype(np.float32),
             "mlp_b2": w["mlp_b2"].astype(np.float32).reshape(1, -1)}
        for b in ("td", "bu"):
            bb = br[b]
            m[f"x2T{b}"] = x2T[b]
            m[f"w2a{b}"] = np.ascontiguousarray(w[f"{b}_w2"][:P].astype(np.float32))
            m[f"w2b{b}"] = np.ascontiguousarray(w[f"{b}_w2"][P:].astype(np.float32))
            m[f"b2{b}"] = w[f"{b}_b2"].astype(np.float32).reshape(1, P)
            m[f"dinvp{b}"] = bb["dinv_part"]
            m[f"dinvl{b}"] = bb["dinv_loc"][c]
            m[f"IDX{b}"] = bb["packed"][c]["IDX16"]
            m[f"DSTL{b}"] = bb["packed"][c]["DSTL"]
            m[f"rx2T{b}"] = rootx2T[b][c]
        maps.append(m)
    return maps


def assemble_x2(pp, results, b):
    N, meta = pp["N"], pp["meta"]
    ns, g0 = meta["node_start"], meta["g0"]
    x2 = np.zeros((N, P), np.float32)
    for c in range(N_CORES):
        lo, hi = int(ns[g0[c]]), int(ns[g0[c + 1]])
        x2[lo:hi] = results[c][f"x2{b}"][: hi - lo]
    return x2


def make_rootx2T(pp, x2, rootindex):
    """Per-core [128, 128] (padded from G_LOC) transposed root features."""
    meta = pp["meta"]
    G_LOC = meta["G_LOC"]
    root = np.asarray(rootindex).astype(np.int64)
    percore = []
    for c in range(N_CORES):
        gi = np.minimum(meta["g0"][c] + np.arange(G_LOC), pp["B"] - 1)
        rt = np.zeros((P, P), np.float32)
        rt[:, :G_LOC] = x2[root[gi]].T
        percore.append(rt)
    return percore


def _run(nc, in_maps):
    return run_bass_kernel_spmd(nc, in_maps, core_ids=list(range(N_CORES))).results


def kernel(x, x_da, edge_index, batch, rootindex,
           td_w1, td_b1, td_w2, td_b2,
           bu_w1, bu_b1, bu_w2, bu_b2,
           mlp_w1, mlp_b1, mlp_w2, mlp_b2):
    w = {"td_w1": td_w1, "td_b1": td_b1, "td_w2": td_w2, "td_b2": td_b2,
         "bu_w1": bu_w1, "bu_b1": bu_b1, "bu_w2": bu_w2, "bu_b2": bu_b2,
         "mlp_w1": mlp_w1, "mlp_b1": mlp_b1, "mlp_w2": mlp_w2, "mlp_b2": mlp_b2}
    w = {k: np.asarray(v) for k, v in w.items()}
    pp = preprocess(np.asarray(x), np.asarray(x_da), np.asarray(edge_index),
                    np.asarray(batch), np.asarray(rootindex))

    nc1 = build_l1(pp)
    res1 = _run(nc1, l1_in_maps(pp, w))

    N, TBL, meta = pp["N"], pp["TBL"], pp["meta"]
    x2T, rootx2T = {}, {}
    for b in ("td", "bu"):
        x2 = assemble_x2(pp, res1, b)
        xt = np.zeros((P, TBL * P), np.float32)
        xt[:, :N] = x2.T
        x2T[b] = xt
        rootx2T[b] = make_rootx2T(pp, x2, rootindex)

    nc2 = build_l2(pp)
    res2 = _run(nc2, l2_in_maps(pp, w, x2T, rootx2T))

    B = pp["B"]
    out = np.zeros((B, 2), np.float32)
    for c in range(N_CORES):
        g0, g1 = meta["g0"][c], meta["g0"][c + 1]
        out[g0:g1] = res2[c]["out"][: g1 - g0]
    return out



# revision 13
# speedup vs baseline: 1.3765x; 1.3765x over previous
"""BiGCN (two-branch GCN + root-extend + scatter-mean + MLP) on 8 trn2 NeuronCores.

Sharding: nodes/edges are sharded by destination across 8 cores using
graph-aligned windows (so scatter-mean pooling stays core-local); the small
weight matrices are replicated. Two SPMD launches (layer-1, then
layer-2+pool+MLP) with host reassembly of layer-1 activations in between.

Per conv layer on device: build the full normalized feature table
ht = dinv * (act @ W) in DRAM (node-major, bf16), then per 128-dst-node tile:
indirect-DMA gather of ht[src] messages (bf16, queue-rotated across the 4
SWDGE queues), one-hot A01 = (dstlocal == iota) built on the vector engine,
PE bf16 matmul segment-sum into PSUM. The GCN bias is folded in as an extra
rank-1 "edge" (outer(sqrt(deg), b)) so the PSUM evacuation is a single
scalar-engine activation (scale by dinv, optional relu). Layer 2's
root-extend term  sum_e dinv[s]*R[batch[s]]  is precomputed on the host as a
dense [B_PAD, NLOC] matrix C and folded into the same PSUM accumulation via
4 extra matmuls per tile against the SBUF-resident R = relu(x0[root]) @ W2b.
"""
import os

import numpy as np

import concourse.bacc as bacc
import concourse.mybir as mybir
import concourse.tile as tile
from concourse.bass_utils import run_bass_kernel_spmd

P = 128
N_CORES = 8
F32 = mybir.dt.float32
BF16 = mybir.dt.bfloat16
BF = mybir.dt.np(mybir.dt.bfloat16)
I16 = mybir.dt.int16


# ----------------------------------------------------------------------------
# host-side preprocessing (index manipulation only)
# ----------------------------------------------------------------------------

def _ceil(a, b):
    return -(-a // b)


def _shard_meta(batch, B, N):
    """Graph-aligned per-core node windows, padded to a uniform 128-aligned
    size. Core c owns graphs [g0[c], g0[c+1]); it computes a window of NLOC
    nodes starting at its first owned node (covering all owned graphs plus a
    partial tail that is discarded)."""
    node_start = np.searchsorted(batch, np.arange(B + 1))
    g0 = [int(_ceil(B * c, N_CORES)) for c in range(N_CORES + 1)]
    spans = [int(node_start[g0[c + 1]] - node_start[g0[c]]) for c in range(N_CORES)]
    NLOC = _ceil(max(spans), P) * P
    T = NLOC // P
    n0 = [int(node_start[g0[c]]) for c in range(N_CORES)]
    gcounts = []
    for c in range(N_CORES):
        hi = min(n0[c] + NLOC, N)
        glast = int(batch[hi - 1]) if hi > n0[c] else g0[c]
        gcounts.append(glast - g0[c] + 1)
    G_LOC = max(gcounts)
    assert G_LOC <= P, f"G_LOC={G_LOC} exceeds 128 partitions"
    return {"node_start": node_start, "g0": g0, "n0": n0, "NLOC": NLOC,
            "T": T, "G_LOC": G_LOC}


CHK = 32768  # dma_gather table-chunk rows (int16 index range)


def _edges_for_core(src, dst, n0, NLOC, N, T, NQ):
    """Edges with dst in this core's window PLUS one self-edge per real
    window node (the GCN self-loop has exactly the edge normalization
    dinv_d*dinv_d, so it is just an extra (d, d) edge). Sorted by
    (dst tile, src); per-(tile, src-chunk) counts."""
    lo, hi = n0, min(n0 + NLOC, N)
    m = (dst >= lo) & (dst < hi)
    es = src[m].astype(np.int64)
    ed = (dst[m] - lo).astype(np.int64)
    sl = np.arange(lo, hi, dtype=np.int64)
    es = np.concatenate([es, sl])
    ed = np.concatenate([ed, sl - lo])
    tl = ed >> 7
    order = np.lexsort((es, tl))
    es, ed, tl = es[order], ed[order], tl[order]
    q = es >> 15
    cnt_tq = np.bincount(tl * NQ + q, minlength=T * NQ).reshape(T, NQ)
    return es, ed, tl, q, cnt_tq


TG = int(os.environ.get("K_TG", "4"))  # dst tiles per merged gather group


def _pack_edges(branch_cores, T, NQ):
    """Union-max per-(tile, chunk) block counts sb[t][q]. Blocks are laid out
    group-major — for each group of TG tiles: for each chunk q: tiles in
    order — so one dma_gather covers a whole (group, chunk) run. Per-core
    padded arrays: IDX16 [128, Mbar*8] int16 (dma_gather wrapped layout, idx
    relative to chunk, pad=0) and DSTL [128, Mbar] bf16 (pad=-1). Flat edge
    slot j of segment (t,q) at block boff[t][q]+j//128, partition j%128 —
    exactly dma_gather's output layout."""
    sb = np.stack([(c["cnt_tq"] + P - 1) // P for c in branch_cores]).max(axis=0)
    ngroups = _ceil(T, TG)
    boff = np.zeros((T, NQ), int)
    goff = np.zeros(ngroups + 1, int)
    gb = np.zeros((ngroups, NQ), int)
    cum = 0
    for g in range(ngroups):
        goff[g] = cum
        for q in range(NQ):
            for t in range(g * TG, min((g + 1) * TG, T)):
                boff[t][q] = cum
                cum += sb[t][q]
                gb[g][q] += sb[t][q]
    goff[ngroups] = cum
    mb = sb.sum(axis=1)
    Mbar = max(1, int(cum))
    out = []
    for c in branch_cores:
        F = np.zeros(Mbar * P, np.int16)
        DSTL = np.full((P, Mbar), -1.0, BF)
        es, ed, tl, q, cnt_tq = (c["es"], c["ed"], c["tl"], c["q"], c["cnt_tq"])
        if len(ed):
            segid = tl * NQ + q
            starts = np.concatenate([[0], np.cumsum(cnt_tq.ravel())])
            within = np.arange(len(ed)) - starts[segid]
            flat = boff.ravel()[segid] * P + within
            F[flat] = (es & (CHK - 1)).astype(np.int16)
            DSTL[flat & 127, flat >> 7] = (ed - (tl << 7)).astype(np.float32)
        IDX16 = np.ascontiguousarray(np.tile(F.reshape(-1, 16).T, (8, 1)))
        out.append({"IDX16": IDX16, "DSTL": DSTL})
    return (sb.astype(int), boff.astype(int), mb.astype(int).tolist(),
            goff.astype(int), gb.astype(int), Mbar, out)


def _part_major(vec, TB, fill):
    v = np.full(TB * P, fill, vec.dtype)
    v[: len(vec)] = vec
    return np.ascontiguousarray(v.reshape(TB, P).T)


def preprocess(x, x_da, edge_index, batch, rootindex):
    import scipy.sparse as sp
    N = x.shape[0]
    B = rootindex.shape[0]
    x0 = np.concatenate([x, x_da], axis=1).astype(np.float32)
    assert x0.shape[1] == P
    TBL = _ceil(N, P)
    x0T = np.zeros((P, TBL * P), BF)
    x0T[:, :N] = x0.T.astype(BF)
    batch = batch.astype(np.int64)
    meta = _shard_meta(batch, B, N)
    T, NLOC, G_LOC = meta["T"], meta["NLOC"], meta["G_LOC"]
    B_PAD = _ceil(B, P) * P

    src_g = edge_index[0].astype(np.int64)
    dst_g = edge_index[1].astype(np.int64)

    NQ = _ceil(TBL * P, CHK)
    branches = {}
    for name, (s, d) in {"td": (src_g, dst_g), "bu": (dst_g, src_g)}.items():
        deg = (np.bincount(d, minlength=N) + 1.0).astype(np.float64)
        dinv = (1.0 / np.sqrt(deg)).astype(np.float32)
        sqdeg = np.sqrt(deg).astype(np.float32)
        cores = []
        for c in range(N_CORES):
            es, ed, tl, q, cnt_tq = _edges_for_core(
                s, d, meta["n0"][c], NLOC, N, T, NQ)
            cores.append({"es": es, "ed": ed, "tl": tl, "q": q,
                          "cnt_tq": cnt_tq})
        sb, boff, mb, goff, gb, Mbar, packed = _pack_edges(cores, T, NQ)
        bd = {"dinv": dinv, "sb": sb, "boff": boff, "mbar": mb,
              "goff": goff, "gb": gb, "Mbar": Mbar, "packed": packed,
              "dinv_part": _part_major(dinv, TBL, np.float32(1.0))}
        loc_dinv, loc_sqdeg, loc_C = [], [], []
        for c in range(N_CORES):
            rows = meta["n0"][c] + np.arange(NLOC)
            valid = rows < N
            rr = np.minimum(rows, N - 1)
            dv = np.where(valid, dinv[rr], 1.0).astype(np.float32)
            loc_dinv.append(np.ascontiguousarray(dv.reshape(T, P).T))
            sq = np.where(valid, sqdeg[rr], 0.0).astype(BF)
            loc_sqdeg.append(np.ascontiguousarray(sq.reshape(1, NLOC)))
            # C[g, dloc] = sum_{e: s->d, d local} dinv[s] * [batch[s] == g]
            # (+ self-loop dinv[d] at batch[d]) — layer-2 root-extend operand.
            ec = cores[c]
            lo = meta["n0"][c]
            real = ec["es"] != (ec["ed"] + lo)  # non-self edges
            gsrc = batch[ec["es"]]
            w = dinv[ec["es"]]
            Cm = sp.coo_matrix((w, (gsrc, ec["ed"])), shape=(B_PAD, NLOC)).toarray()
            loc_C.append(np.ascontiguousarray(Cm.astype(BF)))
        bd["dinv_loc"] = loc_dinv
        bd["sqdeg_loc"] = loc_sqdeg
        bd["C"] = loc_C
        branches[name] = bd

    GL, cinv = [], []
    for c in range(N_CORES):
        rows = meta["n0"][c] + np.arange(NLOC)
        valid = rows < N
        rr = np.minimum(rows, N - 1)
        gl = np.where(valid, batch[rr] - meta["g0"][c], -1).astype(np.float32)
        gl = np.where(gl < G_LOC, gl, -1.0).astype(np.float32)
        GL.append(np.ascontiguousarray(gl.reshape(T, P).T.astype(BF)))
        cnts = np.ones(P, np.float32)
        ns = meta["node_start"]
        for j in range(G_LOC):
            g = meta["g0"][c] + j
            if g < B:
                cc = float(ns[g + 1] - ns[g])
                cnts[j] = cc if cc > 0 else 1.0
        cinv.append((1.0 / cnts).reshape(P, 1).astype(np.float32))

    rootx0T = np.zeros((P, B_PAD), BF)
    rootx0T[:, :B] = np.maximum(x0[rootindex.astype(np.int64)], 0.0).T.astype(BF)

    iota = np.broadcast_to(np.arange(P, dtype=np.float32), (P, P)).astype(BF).copy()
    ident = np.eye(P, dtype=np.float32)

    return {"N": N, "B": B, "TBL": TBL, "B_PAD": B_PAD, "NQ": NQ,
            "meta": meta, "x0": x0, "x0T": x0T, "branches": branches,
            "GL": GL, "cinv": cinv, "rootx0T": rootx0T,
            "iota": iota, "ident": ident}


# ----------------------------------------------------------------------------
# device program builders
# ----------------------------------------------------------------------------

import os
N_QUEUES = int(os.environ.get("K_QUEUES", "4"))
K_SP = os.environ.get("K_SP", "1") == "1"   # single_packet for gathers
K_ROT = os.environ.get("K_ROT", "0") == "1"  # rotate gathers across queues (UNSAFE: sem-lane/queue ordering)
_qctr = [0]


def _next_q():
    if not K_ROT:
        return 0
    q = _qctr[0] % N_QUEUES
    _qctr[0] += 1
    return q


def _new_nc():
    return bacc.Bacc("TRN2", target_bir_lowering=False, debug=False,
                     num_devices=N_CORES, num_swdge_queues=N_QUEUES)


def _load(nc, pool, dram_ap, shape, dtype, tag, bufs=1):
    t = pool.tile(list(shape), dtype, tag=tag, bufs=bufs)
    nc.sync.dma_start(out=t[:], in_=dram_ap)
    return t


def _aggregate(nc, pool, psum, table, IDX_sb, DSTL_sb, bmeta, TROWS,
               iota_sb, T, pre, npre, consume):
    """Group-merged segment-sum: per group of TG dst tiles, one dma_gather
    per 32k-row table chunk fetches all the group's messages, one vector op
    builds all its one-hot blocks, then per tile: PSUM_t = pre-matmuls +
    sum_e A01 . msg (self-loops are real edges) and consume(t, ps) finishes
    (scale/relu/pool/write)."""
    sb, boff, mbar = bmeta["sb"], bmeta["boff"], bmeta["mbar"]
    goff, gb = bmeta["goff"], bmeta["gb"]
    NQ = sb.shape[1]
    ngroups = len(gb)
    gsmax = max(1, int((goff[1:] - goff[:-1]).max()))
    for g in range(ngroups):
        gs = int(goff[g + 1] - goff[g])
        if gs == 0:
            continue  # window tail beyond N: no nodes, nothing to write
        msg = pool.tile([P, gsmax * P], BF16, tag="msg", bufs=2)
        col = 0
        for q in range(NQ):
            nb = int(gb[g][q])
            if nb == 0:
                continue
            base = q * CHK
            rows = min(CHK, TROWS - base)
            b0 = int(goff[g]) + col
            nc.gpsimd.dma_gather(
                out_ap=msg[:, col * P: (col + nb) * P]
                .rearrange("p (b f) -> p b f", f=P),
                in_ap=table[base: base + rows, :],
                idxs_ap=IDX_sb[:, b0 * 8: (b0 + nb) * 8],
                num_idxs=nb * P, num_idxs_reg=nb * P, elem_size=P,
                queue_num=_next_q(), single_packet=K_SP)
            col += nb
        a01 = pool.tile([P, gsmax * P], BF16, tag="a01", bufs=2)
        nc.vector.tensor_tensor(
            out=a01[:, : gs * P].rearrange("p (k f) -> p k f", f=P),
            in0=DSTL_sb[:, goff[g]: goff[g] + gs].to_broadcast([P, gs, P]),
            in1=iota_sb[:].unsqueeze(1).broadcast_to([P, gs, P]),
            op=mybir.AluOpType.is_equal,
        )
        for t in range(g * TG, min((g + 1) * TG, T)):
            if mbar[t] == 0:
                continue
            ps = psum.tile([P, P], F32, tag="ps")
            pre(t, ps)  # npre matmuls, first with start=True
            nk, klast = 0, mbar[t] - 1
            for q in range(NQ):
                for j in range(int(sb[t][q])):
                    k = int(boff[t][q]) - int(goff[g]) + j
                    nc.tensor.matmul(ps[:], lhsT=a01[:, k * P: (k + 1) * P],
                                     rhs=msg[:, k * P: (k + 1) * P],
                                     start=(npre == 0 and nk == 0),
                                     stop=(nk == klast))
                    nk += 1
            consume(t, ps)


def build_l1(pp, reps=1):
    _qctr[0] = 0
    TBL, T = pp["TBL"], pp["meta"]["T"]
    NLOC = pp["meta"]["NLOC"]
    br = pp["branches"]
    nc = _new_nc()
    x0T = nc.dram_tensor("x0T", [P, TBL * P], BF16, kind="ExternalInput")
    iota = nc.dram_tensor("iota", [P, P], BF16, kind="ExternalInput")
    w12 = nc.dram_tensor("w12", [P, 2 * P], BF16, kind="ExternalInput")
    ins = {}
    for b in ("td", "bu"):
        M = br[b]["Mbar"]
        ins[b] = {
            "b1": nc.dram_tensor(f"b1{b}", [1, P], BF16, kind="ExternalInput"),
            "dinv_part": nc.dram_tensor(f"dinvp{b}", [P, TBL], F32, kind="ExternalInput"),
            "dinv_loc": nc.dram_tensor(f"dinvl{b}", [P, T], F32, kind="ExternalInput"),
            "sqdeg": nc.dram_tensor(f"sqdeg{b}", [1, NLOC], BF16, kind="ExternalInput"),
            "IDX": nc.dram_tensor(f"IDX{b}", [P, M * 8], I16, kind="ExternalInput"),
            "DSTL": nc.dram_tensor(f"DSTL{b}", [P, M], BF16, kind="ExternalInput"),
            "table": nc.dram_tensor(f"table{b}", [TBL * P, P], BF16, kind="Internal"),
            "x2": nc.dram_tensor(f"x2{b}", [T * P, P], F32, kind="ExternalOutput"),
        }

    with tile.TileContext(nc) as tc:
        with (
            tc.tile_pool(name="sbuf", bufs=2) as pool,
            tc.tile_pool(name="cst", bufs=1) as cst,
            tc.tile_pool(name="psum", bufs=4, space="PSUM") as psum,
        ):
            iota_sb = _load(nc, cst, iota[:], (P, P), BF16, "iota")
            w12_sb = _load(nc, cst, w12[:], (P, 2 * P), BF16, "w12")
            dinvp_sb = {b: _load(nc, cst, ins[b]["dinv_part"][:], (P, TBL), F32,
                                 f"dinvp{b}") for b in ("td", "bu")}

            import contextlib
            loop_ctx = tc.For_i(0, reps, 1) if reps > 1 else contextlib.nullcontext()
            with loop_ctx:
                _build_l1_body(nc, x0T, pool, cst, psum, pp, ins, iota_sb,
                               w12_sb, dinvp_sb)
    nc.compile()
    return nc


def _build_l1_body(nc, x0T, pool, cst, psum, pp, ins, iota_sb, w12_sb,
                   dinvp_sb):
    TBL, T = pp["TBL"], pp["meta"]["T"]
    br = pp["branches"]
    # ---- tables: ht_b = dinv_b * (x0 @ W1_b), node-major bf16 in DRAM ----
    CH = 8
    for c0 in range(0, TBL, CH):
        nb = min(CH, TBL - c0)
        xt = pool.tile([P, CH * P], BF16, tag="xt", bufs=2)
        nc.sync.dma_start(out=xt[:, : nb * P],
                          in_=x0T[:, c0 * P: (c0 + nb) * P])
        st = {b: pool.tile([P, CH * P], BF16, tag=f"st{b}", bufs=2,
                           name=f"st{b}")
              for b in ("td", "bu")}
        for j in range(nb):
            blk = c0 + j
            psx = psum.tile([P, 2 * P], F32, tag="psx")
            nc.tensor.matmul(psx[:], lhsT=xt[:, j * P: (j + 1) * P],
                             rhs=w12_sb[:], start=True, stop=True)
            for bi, b in enumerate(("td", "bu")):
                nc.scalar.activation(
                    out=st[b][:, j * P: (j + 1) * P],
                    in_=psx[:, bi * P: (bi + 1) * P],
                    func=mybir.ActivationFunctionType.Copy,
                    scale=dinvp_sb[b][:, blk: blk + 1])
        for b in ("td", "bu"):
            nc.sync.dma_start(
                out=ins[b]["table"][c0 * P: (c0 + nb) * P, :]
                .rearrange("(j p) f -> p j f", p=P),
                in_=st[b][:, : nb * P].rearrange("p (j f) -> p j f", f=P))

    # ---- aggregation per branch ----
    for b in ("td", "bu"):
        ib = ins[b]
        M = br[b]["Mbar"]
        IDX_sb = _load(nc, pool, ib["IDX"][:], (P, M * 8), I16, "idx")
        DSTL_sb = _load(nc, pool, ib["DSTL"][:], (P, M), BF16, "dstl")
        dinvl_sb = _load(nc, cst, ib["dinv_loc"][:], (P, T), F32, f"dinvl{b}")
        sqdeg_sb = _load(nc, cst, ib["sqdeg"][:], (1, T * P), BF16, f"sqdeg{b}")
        b1_sb = _load(nc, cst, ib["b1"][:], (1, P), BF16, f"b1{b}")

        def pre(t, ps, sqdeg_sb=sqdeg_sb, b1_sb=b1_sb):
            nc.tensor.matmul(ps[:], lhsT=sqdeg_sb[0:1, t * P: (t + 1) * P],
                             rhs=b1_sb[0:1, :], start=True, stop=False)

        def consume(t, ps, ib=ib, dinvl_sb=dinvl_sb):
            xo = pool.tile([P, P], F32, tag="xo", bufs=3)
            nc.scalar.activation(out=xo[:], in_=ps[:],
                                 func=mybir.ActivationFunctionType.Copy,
                                 scale=dinvl_sb[:, t: t + 1])
            nc.sync.dma_start(out=ib["x2"][t * P: (t + 1) * P, :], in_=xo[:])

        _aggregate(nc, pool, psum, ib["table"], IDX_sb, DSTL_sb,
                   br[b], TBL * P, iota_sb, T, pre, 1, consume)


def build_l2(pp, reps=1):
    _qctr[0] = 0
    TBL, T = pp["TBL"], pp["meta"]["T"]
    G_LOC, B_PAD = pp["meta"]["G_LOC"], pp["B_PAD"]
    NLOC = pp["meta"]["NLOC"]
    br = pp["branches"]
    nc = _new_nc()
    iota = nc.dram_tensor("iota", [P, P], BF16, kind="ExternalInput")
    ident = nc.dram_tensor("ident", [P, P], F32, kind="ExternalInput")
    GL = nc.dram_tensor("GL", [P, T], BF16, kind="ExternalInput")
    cinv = nc.dram_tensor("cinv", [P, 1], F32, kind="ExternalInput")
    rootx0T = nc.dram_tensor("rootx0T", [P, B_PAD], BF16, kind="ExternalInput")
    ones_row = nc.dram_tensor("ones_row", [1, P], F32, kind="ExternalInput")
    mlp_w1 = nc.dram_tensor("mlp_w1", [4 * P, 2 * P], F32, kind="ExternalInput")
    mlp_b1 = nc.dram_tensor("mlp_b1", [1, 2 * P], F32, kind="ExternalInput")
    mlp_w2 = nc.dram_tensor("mlp_w2", [2 * P, 2], F32, kind="ExternalInput")
    mlp_b2 = nc.dram_tensor("mlp_b2", [1, 2], F32, kind="ExternalInput")
    out = nc.dram_tensor("out", [P, 2], F32, kind="ExternalOutput")
    NCH = B_PAD // P  # 4 root/graph chunks
    ins = {}
    for b in ("td", "bu"):
        M = br[b]["Mbar"]
        ins[b] = {
            "x2rT": nc.dram_tensor(f"x2rT{b}", [P, TBL * P], BF16, kind="ExternalInput"),
            "w2a": nc.dram_tensor(f"w2a{b}", [P, P], BF16, kind="ExternalInput"),
            "w2b": nc.dram_tensor(f"w2b{b}", [P, P], BF16, kind="ExternalInput"),
            "b2": nc.dram_tensor(f"b2{b}", [1, P], BF16, kind="ExternalInput"),
            "dinv_part": nc.dram_tensor(f"dinvp{b}", [P, TBL], F32, kind="ExternalInput"),
            "dinv_loc": nc.dram_tensor(f"dinvl{b}", [P, T], F32, kind="ExternalInput"),
            "sqdeg": nc.dram_tensor(f"sqdeg{b}", [1, NLOC], BF16, kind="ExternalInput"),
            "C": nc.dram_tensor(f"C{b}", [B_PAD, NLOC], BF16, kind="ExternalInput"),
            "IDX": nc.dram_tensor(f"IDX{b}", [P, M * 8], I16, kind="ExternalInput"),
            "DSTL": nc.dram_tensor(f"DSTL{b}", [P, M], BF16, kind="ExternalInput"),
            "rootx2T": nc.dram_tensor(f"rx2T{b}", [P, P], F32, kind="ExternalInput"),
            "table": nc.dram_tensor(f"table{b}", [TBL * P, P], BF16, kind="Internal"),
        }

    with tile.TileContext(nc) as tc:
        with (
            tc.tile_pool(name="sbuf", bufs=2) as pool,
            tc.tile_pool(name="cst", bufs=1) as cst,
            tc.tile_pool(name="psum", bufs=4, space="PSUM") as psum,
            tc.tile_pool(name="pps", bufs=1, space="PSUM") as pool_ps,
        ):
            iota_sb = _load(nc, cst, iota[:], (P, P), BF16, "iota")
            ident_sb = _load(nc, cst, ident[:], (P, P), F32, "ident")
            ones_sb = _load(nc, cst, ones_row[:], (1, P), F32, "ones")
            GL_sb = _load(nc, cst, GL[:], (P, T), BF16, "GL")
            cinv_sb = _load(nc, cst, cinv[:], (P, 1), F32, "cinv")
            rx0_sb = _load(nc, cst, rootx0T[:], (P, B_PAD), BF16, "rx0")
            w2a_sb = {b: _load(nc, cst, ins[b]["w2a"][:], (P, P), BF16, f"w2a{b}")
                      for b in ("td", "bu")}
            w2b_sb = {b: _load(nc, cst, ins[b]["w2b"][:], (P, P), BF16, f"w2b{b}")
                      for b in ("td", "bu")}
            dinvp_sb = {b: _load(nc, cst, ins[b]["dinv_part"][:], (P, TBL), F32,
                                 f"dinvp{b}") for b in ("td", "bu")}

            import contextlib
            loop_ctx = tc.For_i(0, reps, 1) if reps > 1 else contextlib.nullcontext()
            with loop_ctx:
                _build_l2_body(
                    nc, pool, cst, psum, pool_ps, pp, ins, mlp_w1,
                    mlp_b1, mlp_w2, mlp_b2, out, iota_sb, ident_sb, ones_sb,
                    GL_sb, cinv_sb, rx0_sb, w2a_sb, w2b_sb, dinvp_sb, NCH)
    nc.compile()
    return nc


def _build_l2_body(nc, pool, cst, psum, pool_ps, pp, ins, mlp_w1,
                   mlp_b1, mlp_w2, mlp_b2, out, iota_sb, ident_sb, ones_sb,
                   GL_sb, cinv_sb, rx0_sb, w2a_sb, w2b_sb, dinvp_sb, NCH):
    TBL, T = pp["TBL"], pp["meta"]["T"]
    G_LOC, B_PAD = pp["meta"]["G_LOC"], pp["B_PAD"]
    br = pp["branches"]
    # ---- R_b = relu(x0[roots]) @ W2b_b, SBUF-resident [128g x 128f] x NCH ----
    R_sb = {}
    for b in ("td", "bu"):
        R_sb[b] = cst.tile([P, NCH * P], BF16, tag=f"R{b}", name=f"R{b}")
        for j in range(NCH):
            psr = psum.tile([P, P], F32, tag="ps")
            nc.tensor.matmul(psr[:], lhsT=rx0_sb[:, j * P: (j + 1) * P],
                             rhs=w2b_sb[b][:], start=True, stop=True)
            nc.scalar.activation(out=R_sb[b][:, j * P: (j + 1) * P], in_=psr[:],
                                 func=mybir.ActivationFunctionType.Copy)

    # ---- ht2 tables: dinv_b * (relu(x2_b) @ W2a_b), bf16 in DRAM ----
    CH = 8
    for b in ("td", "bu"):
        ib = ins[b]
        for c0 in range(0, TBL, CH):
            nb = min(CH, TBL - c0)
            xt = pool.tile([P, CH * P], BF16, tag="xt", bufs=2)
            nc.sync.dma_start(out=xt[:, : nb * P],
                              in_=ib["x2rT"][:, c0 * P: (c0 + nb) * P])
            st = pool.tile([P, CH * P], BF16, tag="st", bufs=2)
            for j in range(nb):
                blk = c0 + j
                psx = psum.tile([P, P], F32, tag="ps")
                nc.tensor.matmul(psx[:], lhsT=xt[:, j * P: (j + 1) * P],
                                 rhs=w2a_sb[b][:], start=True, stop=True)
                nc.scalar.activation(
                    out=st[:, j * P: (j + 1) * P], in_=psx[:],
                    func=mybir.ActivationFunctionType.Copy,
                    scale=dinvp_sb[b][:, blk: blk + 1])
            nc.sync.dma_start(
                out=ib["table"][c0 * P: (c0 + nb) * P, :]
                .rearrange("(j p) f -> p j f", p=P),
                in_=st[:, : nb * P].rearrange("p (j f) -> p j f", f=P))

    # ---- aggregation + relu + pooling per branch ----
    pooled = {}
    for b in ("td", "bu"):
        ib = ins[b]
        M = br[b]["Mbar"]
        IDX_sb = _load(nc, pool, ib["IDX"][:], (P, M * 8), I16, "idx")
        DSTL_sb = _load(nc, pool, ib["DSTL"][:], (P, M), BF16, "dstl")
        dinvl_sb = _load(nc, cst, ib["dinv_loc"][:], (P, T), F32, f"dinvl{b}")
        sqdeg_sb = _load(nc, cst, ib["sqdeg"][:], (1, T * P), BF16, f"sqdeg{b}")
        b2_sb = _load(nc, cst, ib["b2"][:], (1, P), BF16, f"b2{b}")
        ps_pool = pool_ps.tile([G_LOC, P], F32, tag=f"pool{b}")
        t_last = max(t for t in range(T) if br[b]["mbar"][t] > 0)

        def pre(t, ps, ib=ib, sqdeg_sb=sqdeg_sb, b2_sb=b2_sb, Rb=R_sb[b]):
            nc.tensor.matmul(ps[:], lhsT=sqdeg_sb[0:1, t * P: (t + 1) * P],
                             rhs=b2_sb[0:1, :], start=True, stop=False)
            ct = pool.tile([P, NCH * P], BF16, tag="ct", bufs=3)
            nc.sync.dma_start(
                out=ct[:].rearrange("g (c d) -> g c d", d=P),
                in_=ib["C"][:, t * P: (t + 1) * P]
                .rearrange("(c g) d -> g c d", g=P))
            for ci in range(NCH):
                nc.tensor.matmul(ps[:], lhsT=ct[:, ci * P: (ci + 1) * P],
                                 rhs=Rb[:, ci * P: (ci + 1) * P],
                                 start=False, stop=False)

        def consume(t, ps, dinvl_sb=dinvl_sb, ps_pool=ps_pool, t_last=t_last):
            h2 = pool.tile([P, P], BF16, tag="h2", bufs=3)
            nc.scalar.activation(out=h2[:], in_=ps[:],
                                 func=mybir.ActivationFunctionType.Relu,
                                 scale=dinvl_sb[:, t: t + 1])
            oh = pool.tile([P, G_LOC], BF16, tag="oh", bufs=3)
            nc.vector.tensor_tensor(
                out=oh[:], in0=GL_sb[:, t: t + 1].to_broadcast([P, G_LOC]),
                in1=iota_sb[:, :G_LOC], op=mybir.AluOpType.is_equal)
            nc.tensor.matmul(ps_pool[:], lhsT=oh[:], rhs=h2[:],
                             start=(t == 0), stop=(t == t_last))

        _aggregate(nc, pool, psum, ib["table"], IDX_sb, DSTL_sb,
                   br[b], TBL * P, iota_sb, T, pre, 1 + NCH, consume)

        meanS = pool.tile([G_LOC, P], F32, tag=f"mean{b}", bufs=1)
        nc.vector.tensor_scalar(
            out=meanS[:], in0=ps_pool[:], scalar1=cinv_sb[:G_LOC, :],
            scalar2=None, op0=mybir.AluOpType.mult)
        pst = psum.tile([P, G_LOC], F32, tag="ps")
        nc.tensor.transpose(out=pst[:], in_=meanS[:],
                            identity=ident_sb[:G_LOC, :G_LOC])
        meanT = pool.tile([P, G_LOC], F32, tag=f"meanT{b}", bufs=1)
        nc.vector.tensor_copy(out=meanT[:], in_=pst[:])
        pooled[b] = meanT

    # ---- final MLP over this core's G_LOC graph slots ----
    rx2_sb = {b: _load(nc, cst, ins[b]["rootx2T"][:, :P], (P, P), F32,
                       f"rx2{b}") for b in ("td", "bu")}
    w1c_sb = cst.tile([P, 4 * 2 * P], F32, tag="mw1", name="mw1")
    nc.sync.dma_start(
        out=w1c_sb[:].rearrange("p (c o) -> p c o", c=4),
        in_=mlp_w1[:].rearrange("(c p) o -> p c o", p=P))
    b1m_sb = _load(nc, cst, mlp_b1[:], (1, 2 * P), F32, "mb1")
    w2c_sb = cst.tile([P, 2 * 2], F32, tag="mw2", name="mw2")
    nc.sync.dma_start(
        out=w2c_sb[:].rearrange("p (c o) -> p c o", c=2),
        in_=mlp_w2[:].rearrange("(c p) o -> p c o", p=P))
    b2m_sb = _load(nc, cst, mlp_b2[:], (1, 2), F32, "mb2")

    ps1 = psum.tile([G_LOC, 2 * P], F32, tag="ps")
    chunks = [pooled["bu"][:, :G_LOC], rx2_sb["bu"][:, :G_LOC],
              pooled["td"][:, :G_LOC], rx2_sb["td"][:, :G_LOC]]
    for ci, lhsT in enumerate(chunks):
        nc.tensor.matmul(ps1[:], lhsT=lhsT,
                         rhs=w1c_sb[:, ci * 2 * P: (ci + 1) * 2 * P],
                         start=(ci == 0), stop=False)
    nc.tensor.matmul(ps1[:], lhsT=ones_sb[0:1, :G_LOC], rhs=b1m_sb[0:1, :],
                     start=False, stop=True)
    h1 = pool.tile([G_LOC, 2 * P], F32, tag="mlph", bufs=1)
    nc.scalar.activation(out=h1[:], in_=ps1[:],
                         func=mybir.ActivationFunctionType.Relu)
    hT = []
    for ci in range(2):
        pst2 = psum.tile([P, G_LOC], F32, tag="ps")
        nc.tensor.transpose(out=pst2[:], in_=h1[:, ci * P: (ci + 1) * P],
                            identity=ident_sb[:G_LOC, :G_LOC])
        ht_sb = pool.tile([P, G_LOC], F32, tag=f"hT{ci}", bufs=1)
        nc.vector.tensor_copy(out=ht_sb[:], in_=pst2[:])
        hT.append(ht_sb)
    ps2 = psum.tile([G_LOC, 2], F32, tag="ps")
    for ci in range(2):
        nc.tensor.matmul(ps2[:], lhsT=hT[ci][:, :G_LOC],
                         rhs=w2c_sb[:, ci * 2: (ci + 1) * 2],
                         start=(ci == 0), stop=False)
    nc.tensor.matmul(ps2[:], lhsT=ones_sb[0:1, :G_LOC], rhs=b2m_sb[0:1, :],
                     start=False, stop=True)
    oo = pool.tile([G_LOC, 2], F32, tag="oo", bufs=1)
    nc.vector.tensor_copy(out=oo[:], in_=ps2[:])
    nc.sync.dma_start(out=out[:G_LOC, :], in_=oo[:])


# ----------------------------------------------------------------------------
# in_map assembly + kernel entry
# ----------------------------------------------------------------------------

def l1_in_maps(pp, w):
    br = pp["branches"]
    w12 = np.concatenate([w["td_w1"], w["bu_w1"]], axis=1).astype(BF)
    maps = []
    for c in range(N_CORES):
        m = {"x0T": pp["x0T"], "iota": pp["iota"],
             "w12": np.ascontiguousarray(w12)}
        for b in ("td", "bu"):
            bb = br[b]
            m[f"b1{b}"] = w[f"{b}_b1"].astype(BF).reshape(1, P)
            m[f"dinvp{b}"] = bb["dinv_part"]
            m[f"dinvl{b}"] = bb["dinv_loc"][c]
            m[f"sqdeg{b}"] = bb["sqdeg_loc"][c]
            m[f"IDX{b}"] = bb["packed"][c]["IDX16"]
            m[f"DSTL{b}"] = bb["packed"][c]["DSTL"]
        maps.append(m)
    return maps


def l2_in_maps(pp, w, x2rT, rootx2T):
    br = pp["branches"]
    ones = np.ones((1, P), np.float32)
    maps = []
    for c in range(N_CORES):
        m = {"iota": pp["iota"], "ident": pp["ident"],
             "GL": pp["GL"][c],
             "cinv": pp["cinv"][c], "rootx0T": pp["rootx0T"], "ones_row": ones,
             "mlp_w1": w["mlp_w1"].astype(np.float32),
             "mlp_b1": w["mlp_b1"].astype(np.float32).reshape(1, -1),
             "mlp_w2": w["mlp_w2"].astype(np.float32),
             "mlp_b2": w["mlp_b2"].astype(np.float32).reshape(1, -1)}
        for b in ("td", "bu"):
            bb = br[b]
            m[f"x2rT{b}"] = x2rT[b]
            m[f"w2a{b}"] = np.ascontiguousarray(w[f"{b}_w2"][:P].astype(BF))
            m[f"w2b{b}"] = np.ascontiguousarray(w[f"{b}_w2"][P:].astype(BF))
            m[f"b2{b}"] = w[f"{b}_b2"].astype(BF).reshape(1, P)
            m[f"dinvp{b}"] = bb["dinv_part"]
            m[f"dinvl{b}"] = bb["dinv_loc"][c]
            m[f"sqdeg{b}"] = bb["sqdeg_loc"][c]
            m[f"C{b}"] = bb["C"][c]
            m[f"IDX{b}"] = bb["packed"][c]["IDX16"]
            m[f"DSTL{b}"] = bb["packed"][c]["DSTL"]
            m[f"rx2T{b}"] = rootx2T[b][c]
        maps.append(m)
    return maps


def assemble_x2(pp, results, b):
    N, meta = pp["N"], pp["meta"]
    ns, g0 = meta["node_start"], meta["g0"]
    x2 = np.zeros((N, P), np.float32)
    for c in range(N_CORES):
        lo, hi = int(ns[g0[c]]), int(ns[g0[c + 1]])
        x2[lo:hi] = results[c][f"x2{b}"][: hi - lo]
    return x2


def make_x2rT(pp, x2):
    """[128, TBL*128] bf16 relu(x2) transposed — layer-2 table-build input."""
    N, TBL = pp["N"], pp["TBL"]
    xt = np.zeros((P, TBL * P), BF)
    xt[:, :N] = np.maximum(x2, 0.0).T.astype(BF)
    return xt


def make_rootx2T(pp, x2, rootindex):
    """Per-core [128, 128] (padded from G_LOC) transposed root features."""
    meta = pp["meta"]
    G_LOC = meta["G_LOC"]
    root = np.asarray(rootindex).astype(np.int64)
    percore = []
    for c in range(N_CORES):
        gi = np.minimum(meta["g0"][c] + np.arange(G_LOC), pp["B"] - 1)
        rt = np.zeros((P, P), np.float32)
        rt[:, :G_LOC] = x2[root[gi]].T
        percore.append(rt)
    return percore


def _run(nc, in_maps):
    return run_bass_kernel_spmd(nc, in_maps, core_ids=list(range(N_CORES))).results


def kernel(x, x_da, edge_index, batch, rootindex,
           td_w1, td_b1, td_w2, td_b2,
           bu_w1, bu_b1, bu_w2, bu_b2,
           mlp_w1, mlp_b1, mlp_w2, mlp_b2):
    w = {"td_w1": td_w1, "td_b1": td_b1, "td_w2": td_w2, "td_b2": td_b2,
         "bu_w1": bu_w1, "bu_b1": bu_b1, "bu_w2": bu_w2, "bu_b2": bu_b2,
         "mlp_w1": mlp_w1, "mlp_b1": mlp_b1, "mlp_w2": mlp_w2, "mlp_b2": mlp_b2}
    w = {k: np.asarray(v) for k, v in w.items()}
    pp = preprocess(np.asarray(x), np.asarray(x_da), np.asarray(edge_index),
                    np.asarray(batch), np.asarray(rootindex))

    nc1 = build_l1(pp)
    res1 = _run(nc1, l1_in_maps(pp, w))

    x2rT, rootx2T = {}, {}
    for b in ("td", "bu"):
        x2 = assemble_x2(pp, res1, b)
        x2rT[b] = make_x2rT(pp, x2)
        rootx2T[b] = make_rootx2T(pp, x2, rootindex)

    nc2 = build_l2(pp)
    res2 = _run(nc2, l2_in_maps(pp, w, x2rT, rootx2T))

    B = pp["B"]
    meta = pp["meta"]
    out = np.zeros((B, 2), np.float32)
    for c in range(N_CORES):
        g0, g1 = meta["g0"][c], meta["g0"][c + 1]
        out[g0:g1] = res2[c]["out"][: g1 - g0]
    return out


# revision 15
# speedup vs baseline: 8.1528x; 5.9231x over previous
"""BiGCN (two-branch GCN + root-extend + scatter-mean + MLP) on 8 trn2 NeuronCores.

Sharding: nodes/edges are sharded by destination across 8 cores using
graph-aligned windows (so scatter-mean pooling stays core-local); the small
weight matrices are replicated. Two SPMD launches (layer-1, then
layer-2+pool+MLP) with host reassembly of layer-1 activations in between.

Per conv layer on device: build the full normalized feature table
ht = dinv * (act @ W) in DRAM (node-major, bf16), then per 128-dst-node tile:
indirect-DMA gather of ht[src] messages (bf16, queue-rotated across the 4
SWDGE queues), one-hot A01 = (dstlocal == iota) built on the vector engine,
PE bf16 matmul segment-sum into PSUM. The GCN bias is folded in as an extra
rank-1 "edge" (outer(sqrt(deg), b)) so the PSUM evacuation is a single
scalar-engine activation (scale by dinv, optional relu). Layer 2's
root-extend term  sum_e dinv[s]*R[batch[s]]  is precomputed on the host as a
dense [B_PAD, NLOC] matrix C and folded into the same PSUM accumulation via
4 extra matmuls per tile against the SBUF-resident R = relu(x0[root]) @ W2b.
"""
import os

import numpy as np

import concourse.bacc as bacc
import concourse.mybir as mybir
import concourse.tile as tile
from concourse.bass_utils import run_bass_kernel_spmd

P = 128
N_CORES = 8
F32 = mybir.dt.float32
BF16 = mybir.dt.bfloat16
BF = mybir.dt.np(mybir.dt.bfloat16)
I16 = mybir.dt.int16


# ----------------------------------------------------------------------------
# host-side preprocessing (index manipulation only)
# ----------------------------------------------------------------------------

def _ceil(a, b):
    return -(-a // b)


def _shard_meta(batch, B, N):
    """Graph-aligned per-core node windows, padded to a uniform 128-aligned
    size. Core c owns graphs [g0[c], g0[c+1]); it computes a window of NLOC
    nodes starting at its first owned node (covering all owned graphs plus a
    partial tail that is discarded)."""
    node_start = np.searchsorted(batch, np.arange(B + 1))
    g0 = [int(_ceil(B * c, N_CORES)) for c in range(N_CORES + 1)]
    spans = [int(node_start[g0[c + 1]] - node_start[g0[c]]) for c in range(N_CORES)]
    NLOC = _ceil(max(spans), P) * P
    T = NLOC // P
    n0 = [int(node_start[g0[c]]) for c in range(N_CORES)]
    gcounts = []
    for c in range(N_CORES):
        hi = min(n0[c] + NLOC, N)
        glast = int(batch[hi - 1]) if hi > n0[c] else g0[c]
        gcounts.append(glast - g0[c] + 1)
    G_LOC = max(gcounts)
    assert G_LOC <= P, f"G_LOC={G_LOC} exceeds 128 partitions"
    return {"node_start": node_start, "g0": g0, "n0": n0, "NLOC": NLOC,
            "T": T, "G_LOC": G_LOC}


CHK = 32768  # dma_gather table-chunk rows (int16 index range)


def _edges_for_core(src, dst, n0, NLOC, N, T, NQ):
    """Edges with dst in this core's window PLUS one self-edge per real
    window node (the GCN self-loop has exactly the edge normalization
    dinv_d*dinv_d, so it is just an extra (d, d) edge). Sorted by
    (dst tile, src); per-(tile, src-chunk) counts."""
    lo, hi = n0, min(n0 + NLOC, N)
    m = (dst >= lo) & (dst < hi)
    es = src[m].astype(np.int64)
    ed = (dst[m] - lo).astype(np.int64)
    sl = np.arange(lo, hi, dtype=np.int64)
    es = np.concatenate([es, sl])
    ed = np.concatenate([ed, sl - lo])
    tl = ed >> 7
    order = np.lexsort((es, tl))
    es, ed, tl = es[order], ed[order], tl[order]
    q = es >> 15
    cnt_tq = np.bincount(tl * NQ + q, minlength=T * NQ).reshape(T, NQ)
    return es, ed, tl, q, cnt_tq


TG = int(os.environ.get("K_TG", "1"))  # dst tiles per merged gather group
# TG>1 merges gather calls across dst tiles; >=2 has crashed HW (SWDGE ring
# carveout overflow for ~3k+ descriptor calls) — keep 1 unless re-validated.


def _pack_edges(branch_cores, T, NQ):
    """Union-max per-(tile, chunk) block counts sb[t][q]. Blocks are laid out
    group-major — for each group of TG tiles: for each chunk q: tiles in
    order — so one dma_gather covers a whole (group, chunk) run. Per-core
    padded arrays: IDX16 [128, Mbar*8] int16 (dma_gather wrapped layout, idx
    relative to chunk, pad=0) and DSTL [128, Mbar] bf16 (pad=-1). Flat edge
    slot j of segment (t,q) at block boff[t][q]+j//128, partition j%128 —
    exactly dma_gather's output layout."""
    sb = np.stack([(c["cnt_tq"] + P - 1) // P for c in branch_cores]).max(axis=0)
    ngroups = _ceil(T, TG)
    boff = np.zeros((T, NQ), int)
    goff = np.zeros(ngroups + 1, int)
    gb = np.zeros((ngroups, NQ), int)
    cum = 0
    for g in range(ngroups):
        goff[g] = cum
        for q in range(NQ):
            for t in range(g * TG, min((g + 1) * TG, T)):
                boff[t][q] = cum
                cum += sb[t][q]
                gb[g][q] += sb[t][q]
    goff[ngroups] = cum
    mb = sb.sum(axis=1)
    Mbar = max(1, int(cum))
    out = []
    for c in branch_cores:
        F = np.zeros(Mbar * P, np.int16)
        DSTL = np.full((P, Mbar), -1.0, BF)
        es, ed, tl, q, cnt_tq = (c["es"], c["ed"], c["tl"], c["q"], c["cnt_tq"])
        if len(ed):
            segid = tl * NQ + q
            starts = np.concatenate([[0], np.cumsum(cnt_tq.ravel())])
            within = np.arange(len(ed)) - starts[segid]
            flat = boff.ravel()[segid] * P + within
            F[flat] = (es & (CHK - 1)).astype(np.int16)
            DSTL[flat & 127, flat >> 7] = (ed - (tl << 7)).astype(np.float32)
        IDX16 = np.ascontiguousarray(np.tile(F.reshape(-1, 16).T, (8, 1)))
        out.append({"IDX16": IDX16, "DSTL": DSTL})
    return (sb.astype(int), boff.astype(int), mb.astype(int).tolist(),
            goff.astype(int), gb.astype(int), Mbar, out)


def _part_major(vec, TB, fill):
    v = np.full(TB * P, fill, vec.dtype)
    v[: len(vec)] = vec
    return np.ascontiguousarray(v.reshape(TB, P).T)


def preprocess(x, x_da, edge_index, batch, rootindex):
    import scipy.sparse as sp
    N = x.shape[0]
    B = rootindex.shape[0]
    x0 = np.concatenate([x, x_da], axis=1).astype(np.float32)
    assert x0.shape[1] == P
    TBL = _ceil(N, P)
    x0T = np.zeros((P, TBL * P), BF)
    x0T[:, :N] = x0.T.astype(BF)
    batch = batch.astype(np.int64)
    meta = _shard_meta(batch, B, N)
    T, NLOC, G_LOC = meta["T"], meta["NLOC"], meta["G_LOC"]
    B_PAD = _ceil(B, P) * P

    src_g = edge_index[0].astype(np.int64)
    dst_g = edge_index[1].astype(np.int64)

    NQ = _ceil(TBL * P, CHK)
    branches = {}
    for name, (s, d) in {"td": (src_g, dst_g), "bu": (dst_g, src_g)}.items():
        deg = (np.bincount(d, minlength=N) + 1.0).astype(np.float64)
        dinv = (1.0 / np.sqrt(deg)).astype(np.float32)
        sqdeg = np.sqrt(deg).astype(np.float32)
        cores = []
        for c in range(N_CORES):
            es, ed, tl, q, cnt_tq = _edges_for_core(
                s, d, meta["n0"][c], NLOC, N, T, NQ)
            cores.append({"es": es, "ed": ed, "tl": tl, "q": q,
                          "cnt_tq": cnt_tq})
        sb, boff, mb, goff, gb, Mbar, packed = _pack_edges(cores, T, NQ)
        bd = {"dinv": dinv, "sb": sb, "boff": boff, "mbar": mb,
              "goff": goff, "gb": gb, "Mbar": Mbar, "packed": packed,
              "dinv_part": _part_major(dinv, TBL, np.float32(1.0))}
        loc_dinv, loc_sqdeg, loc_C = [], [], []
        for c in range(N_CORES):
            rows = meta["n0"][c] + np.arange(NLOC)
            valid = rows < N
            rr = np.minimum(rows, N - 1)
            dv = np.where(valid, dinv[rr], 1.0).astype(np.float32)
            loc_dinv.append(np.ascontiguousarray(dv.reshape(T, P).T))
            sq = np.where(valid, sqdeg[rr], 0.0).astype(BF)
            loc_sqdeg.append(np.ascontiguousarray(sq.reshape(1, NLOC)))
            # C[g, dloc] = sum_{e: s->d, d local} dinv[s] * [batch[s] == g]
            # (+ self-loop dinv[d] at batch[d]) — layer-2 root-extend operand.
            ec = cores[c]
            gsrc = batch[ec["es"]]
            w = dinv[ec["es"]]
            Cm = sp.coo_matrix((w, (gsrc, ec["ed"])), shape=(B_PAD, NLOC)).toarray()
            loc_C.append(np.ascontiguousarray(Cm.astype(BF)))
        bd["dinv_loc"] = loc_dinv
        bd["sqdeg_loc"] = loc_sqdeg
        bd["C"] = loc_C
        branches[name] = bd

    GL, cinv = [], []
    for c in range(N_CORES):
        rows = meta["n0"][c] + np.arange(NLOC)
        valid = rows < N
        rr = np.minimum(rows, N - 1)
        gl = np.where(valid, batch[rr] - meta["g0"][c], -1).astype(np.float32)
        gl = np.where(gl < G_LOC, gl, -1.0).astype(np.float32)
        GL.append(np.ascontiguousarray(gl.reshape(T, P).T.astype(BF)))
        cnts = np.ones(P, np.float32)
        ns = meta["node_start"]
        for j in range(G_LOC):
            g = meta["g0"][c] + j
            if g < B:
                cc = float(ns[g + 1] - ns[g])
                cnts[j] = cc if cc > 0 else 1.0
        cinv.append((1.0 / cnts).reshape(P, 1).astype(np.float32))

    rootx0T = np.zeros((P, B_PAD), BF)
    rootx0T[:, :B] = np.maximum(x0[rootindex.astype(np.int64)], 0.0).T.astype(BF)

    iota = np.broadcast_to(np.arange(P, dtype=np.float32), (P, P)).astype(BF).copy()
    ident = np.eye(P, dtype=np.float32)

    return {"N": N, "B": B, "TBL": TBL, "B_PAD": B_PAD, "NQ": NQ,
            "meta": meta, "x0": x0, "x0T": x0T, "branches": branches,
            "GL": GL, "cinv": cinv, "rootx0T": rootx0T,
            "iota": iota, "ident": ident}


# ----------------------------------------------------------------------------
# device program builders
# ----------------------------------------------------------------------------

import os
N_QUEUES = int(os.environ.get("K_QUEUES", "4"))
K_SP = os.environ.get("K_SP", "1") == "1"   # single_packet for gathers
K_ROT = os.environ.get("K_ROT", "0") == "1"  # rotate gathers across queues (UNSAFE: sem-lane/queue ordering)
_qctr = [0]


def _next_q():
    if not K_ROT:
        return 0
    q = _qctr[0] % N_QUEUES
    _qctr[0] += 1
    return q


def _new_nc():
    return bacc.Bacc("TRN2", target_bir_lowering=False, debug=False,
                     num_devices=N_CORES, num_swdge_queues=N_QUEUES)


def _load(nc, pool, dram_ap, shape, dtype, tag, bufs=1):
    t = pool.tile(list(shape), dtype, tag=tag, bufs=bufs)
    nc.sync.dma_start(out=t[:], in_=dram_ap)
    return t


def _aggregate(nc, pool, psum, table, IDX_sb, DSTL_sb, bmeta, TROWS,
               iota_sb, T, pre, npre, consume):
    """Group-merged segment-sum: per group of TG dst tiles, one dma_gather
    per 32k-row table chunk fetches all the group's messages, one vector op
    builds all its one-hot blocks, then per tile: PSUM_t = pre-matmuls +
    sum_e A01 . msg (self-loops are real edges) and consume(t, ps) finishes
    (scale/relu/pool/write)."""
    sb, boff, mbar = bmeta["sb"], bmeta["boff"], bmeta["mbar"]
    goff, gb = bmeta["goff"], bmeta["gb"]
    NQ = sb.shape[1]
    ngroups = len(gb)
    gsmax = max(1, int((goff[1:] - goff[:-1]).max()))
    for g in range(ngroups):
        gs = int(goff[g + 1] - goff[g])
        if gs == 0:
            continue  # window tail beyond N: no nodes, nothing to write
        msg = pool.tile([P, gsmax * P], BF16, tag="msg", bufs=2)
        col = 0
        for q in range(NQ):
            nb = int(gb[g][q])
            if nb == 0:
                continue
            base = q * CHK
            rows = min(CHK, TROWS - base)
            b0 = int(goff[g]) + col
            nc.gpsimd.dma_gather(
                out_ap=msg[:, col * P: (col + nb) * P]
                .rearrange("p (b f) -> p b f", f=P),
                in_ap=table[base: base + rows, :],
                idxs_ap=IDX_sb[:, b0 * 8: (b0 + nb) * 8],
                num_idxs=nb * P, num_idxs_reg=nb * P, elem_size=P,
                queue_num=_next_q(), single_packet=K_SP)
            col += nb
        a01 = pool.tile([P, gsmax * P], BF16, tag="a01", bufs=2)
        nc.vector.tensor_tensor(
            out=a01[:, : gs * P].rearrange("p (k f) -> p k f", f=P),
            in0=DSTL_sb[:, goff[g]: goff[g] + gs].to_broadcast([P, gs, P]),
            in1=iota_sb[:].unsqueeze(1).broadcast_to([P, gs, P]),
            op=mybir.AluOpType.is_equal,
        )
        for t in range(g * TG, min((g + 1) * TG, T)):
            if mbar[t] == 0:
                continue
            ps = psum.tile([P, P], F32, tag="ps")
            pre(t, ps)  # npre matmuls, first with start=True
            nk, klast = 0, mbar[t] - 1
            for q in range(NQ):
                for j in range(int(sb[t][q])):
                    k = int(boff[t][q]) - int(goff[g]) + j
                    nc.tensor.matmul(ps[:], lhsT=a01[:, k * P: (k + 1) * P],
                                     rhs=msg[:, k * P: (k + 1) * P],
                                     start=(npre == 0 and nk == 0),
                                     stop=(nk == klast))
                    nk += 1
            consume(t, ps)


def build_l1(pp, reps=1):
    _qctr[0] = 0
    TBL, T = pp["TBL"], pp["meta"]["T"]
    NLOC = pp["meta"]["NLOC"]
    br = pp["branches"]
    nc = _new_nc()
    x0T = nc.dram_tensor("x0T", [P, TBL * P], BF16, kind="ExternalInput")
    iota = nc.dram_tensor("iota", [P, P], BF16, kind="ExternalInput")
    w12 = nc.dram_tensor("w12", [P, 2 * P], BF16, kind="ExternalInput")
    ins = {}
    for b in ("td", "bu"):
        M = br[b]["Mbar"]
        ins[b] = {
            "b1": nc.dram_tensor(f"b1{b}", [1, P], BF16, kind="ExternalInput"),
            "dinv_part": nc.dram_tensor(f"dinvp{b}", [P, TBL], F32, kind="ExternalInput"),
            "dinv_loc": nc.dram_tensor(f"dinvl{b}", [P, T], F32, kind="ExternalInput"),
            "sqdeg": nc.dram_tensor(f"sqdeg{b}", [1, NLOC], BF16, kind="ExternalInput"),
            "IDX": nc.dram_tensor(f"IDX{b}", [P, M * 8], I16, kind="ExternalInput"),
            "DSTL": nc.dram_tensor(f"DSTL{b}", [P, M], BF16, kind="ExternalInput"),
            "table": nc.dram_tensor(f"table{b}", [TBL * P, P], BF16, kind="Internal"),
            "x2": nc.dram_tensor(f"x2{b}", [T * P, P], F32, kind="ExternalOutput"),
        }

    with tile.TileContext(nc) as tc:
        with (
            tc.tile_pool(name="sbuf", bufs=2) as pool,
            tc.tile_pool(name="cst", bufs=1) as cst,
            tc.tile_pool(name="psum", bufs=4, space="PSUM") as psum,
        ):
            iota_sb = _load(nc, cst, iota[:], (P, P), BF16, "iota")
            w12_sb = _load(nc, cst, w12[:], (P, 2 * P), BF16, "w12")
            dinvp_sb = {b: _load(nc, cst, ins[b]["dinv_part"][:], (P, TBL), F32,
                                 f"dinvp{b}") for b in ("td", "bu")}

            import contextlib
            loop_ctx = tc.For_i(0, reps, 1) if reps > 1 else contextlib.nullcontext()
            with loop_ctx:
                _build_l1_body(nc, x0T, pool, cst, psum, pp, ins, iota_sb,
                               w12_sb, dinvp_sb)
    nc.compile()
    return nc


def _build_l1_body(nc, x0T, pool, cst, psum, pp, ins, iota_sb, w12_sb,
                   dinvp_sb):
    TBL, T = pp["TBL"], pp["meta"]["T"]
    br = pp["branches"]
    # ---- tables: ht_b = dinv_b * (x0 @ W1_b), node-major bf16 in DRAM ----
    CH = 8
    for c0 in range(0, TBL, CH):
        nb = min(CH, TBL - c0)
        xt = pool.tile([P, CH * P], BF16, tag="xt", bufs=2)
        nc.sync.dma_start(out=xt[:, : nb * P],
                          in_=x0T[:, c0 * P: (c0 + nb) * P])
        st = {b: pool.tile([P, CH * P], BF16, tag=f"st{b}", bufs=2,
                           name=f"st{b}")
              for b in ("td", "bu")}
        for j in range(nb):
            blk = c0 + j
            psx = psum.tile([P, 2 * P], F32, tag="psx")
            nc.tensor.matmul(psx[:], lhsT=xt[:, j * P: (j + 1) * P],
                             rhs=w12_sb[:], start=True, stop=True)
            for bi, b in enumerate(("td", "bu")):
                nc.scalar.activation(
                    out=st[b][:, j * P: (j + 1) * P],
                    in_=psx[:, bi * P: (bi + 1) * P],
                    func=mybir.ActivationFunctionType.Copy,
                    scale=dinvp_sb[b][:, blk: blk + 1])
        for b in ("td", "bu"):
            nc.sync.dma_start(
                out=ins[b]["table"][c0 * P: (c0 + nb) * P, :]
                .rearrange("(j p) f -> p j f", p=P),
                in_=st[b][:, : nb * P].rearrange("p (j f) -> p j f", f=P))

    # ---- aggregation per branch ----
    for b in ("td", "bu"):
        ib = ins[b]
        M = br[b]["Mbar"]
        IDX_sb = _load(nc, pool, ib["IDX"][:], (P, M * 8), I16, "idx")
        DSTL_sb = _load(nc, pool, ib["DSTL"][:], (P, M), BF16, "dstl")
        dinvl_sb = _load(nc, cst, ib["dinv_loc"][:], (P, T), F32, f"dinvl{b}")
        sqdeg_sb = _load(nc, cst, ib["sqdeg"][:], (1, T * P), BF16, f"sqdeg{b}")
        b1_sb = _load(nc, cst, ib["b1"][:], (1, P), BF16, f"b1{b}")

        def pre(t, ps, sqdeg_sb=sqdeg_sb, b1_sb=b1_sb):
            nc.tensor.matmul(ps[:], lhsT=sqdeg_sb[0:1, t * P: (t + 1) * P],
                             rhs=b1_sb[0:1, :], start=True, stop=False)

        def consume(t, ps, ib=ib, dinvl_sb=dinvl_sb):
            xo = pool.tile([P, P], F32, tag="xo", bufs=3)
            nc.scalar.activation(out=xo[:], in_=ps[:],
                                 func=mybir.ActivationFunctionType.Copy,
                                 scale=dinvl_sb[:, t: t + 1])
            nc.sync.dma_start(out=ib["x2"][t * P: (t + 1) * P, :], in_=xo[:])

        _aggregate(nc, pool, psum, ib["table"], IDX_sb, DSTL_sb,
                   br[b], TBL * P, iota_sb, T, pre, 1, consume)


def build_l2(pp, reps=1):
    _qctr[0] = 0
    TBL, T = pp["TBL"], pp["meta"]["T"]
    G_LOC, B_PAD = pp["meta"]["G_LOC"], pp["B_PAD"]
    NLOC = pp["meta"]["NLOC"]
    br = pp["branches"]
    nc = _new_nc()
    iota = nc.dram_tensor("iota", [P, P], BF16, kind="ExternalInput")
    ident = nc.dram_tensor("ident", [P, P], F32, kind="ExternalInput")
    GL = nc.dram_tensor("GL", [P, T], BF16, kind="ExternalInput")
    cinv = nc.dram_tensor("cinv", [P, 1], F32, kind="ExternalInput")
    rootx0T = nc.dram_tensor("rootx0T", [P, B_PAD], BF16, kind="ExternalInput")
    ones_row = nc.dram_tensor("ones_row", [1, P], F32, kind="ExternalInput")
    mlp_w1 = nc.dram_tensor("mlp_w1", [4 * P, 2 * P], F32, kind="ExternalInput")
    mlp_b1 = nc.dram_tensor("mlp_b1", [1, 2 * P], F32, kind="ExternalInput")
    mlp_w2 = nc.dram_tensor("mlp_w2", [2 * P, 2], F32, kind="ExternalInput")
    mlp_b2 = nc.dram_tensor("mlp_b2", [1, 2], F32, kind="ExternalInput")
    out = nc.dram_tensor("out", [P, 2], F32, kind="ExternalOutput")
    NCH = B_PAD // P  # 4 root/graph chunks
    ins = {}
    for b in ("td", "bu"):
        M = br[b]["Mbar"]
        ins[b] = {
            "x2rT": nc.dram_tensor(f"x2rT{b}", [P, TBL * P], BF16, kind="ExternalInput"),
            "w2a": nc.dram_tensor(f"w2a{b}", [P, P], BF16, kind="ExternalInput"),
            "w2b": nc.dram_tensor(f"w2b{b}", [P, P], BF16, kind="ExternalInput"),
            "b2": nc.dram_tensor(f"b2{b}", [1, P], BF16, kind="ExternalInput"),
            "dinv_part": nc.dram_tensor(f"dinvp{b}", [P, TBL], F32, kind="ExternalInput"),
            "dinv_loc": nc.dram_tensor(f"dinvl{b}", [P, T], F32, kind="ExternalInput"),
            "sqdeg": nc.dram_tensor(f"sqdeg{b}", [1, NLOC], BF16, kind="ExternalInput"),
            "C": nc.dram_tensor(f"C{b}", [B_PAD, NLOC], BF16, kind="ExternalInput"),
            "IDX": nc.dram_tensor(f"IDX{b}", [P, M * 8], I16, kind="ExternalInput"),
            "DSTL": nc.dram_tensor(f"DSTL{b}", [P, M], BF16, kind="ExternalInput"),
            "rootx2T": nc.dram_tensor(f"rx2T{b}", [P, P], F32, kind="ExternalInput"),
            "table": nc.dram_tensor(f"table{b}", [TBL * P, P], BF16, kind="Internal"),
        }

    with tile.TileContext(nc) as tc:
        with (
            tc.tile_pool(name="sbuf", bufs=2) as pool,
            tc.tile_pool(name="cst", bufs=1) as cst,
            tc.tile_pool(name="psum", bufs=4, space="PSUM") as psum,
            tc.tile_pool(name="pps", bufs=1, space="PSUM") as pool_ps,
        ):
            iota_sb = _load(nc, cst, iota[:], (P, P), BF16, "iota")
            ident_sb = _load(nc, cst, ident[:], (P, P), F32, "ident")
            ones_sb = _load(nc, cst, ones_row[:], (1, P), F32, "ones")
            GL_sb = _load(nc, cst, GL[:], (P, T), BF16, "GL")
            cinv_sb = _load(nc, cst, cinv[:], (P, 1), F32, "cinv")
            rx0_sb = _load(nc, cst, rootx0T[:], (P, B_PAD), BF16, "rx0")
            w2a_sb = {b: _load(nc, cst, ins[b]["w2a"][:], (P, P), BF16, f"w2a{b}")
                      for b in ("td", "bu")}
            w2b_sb = {b: _load(nc, cst, ins[b]["w2b"][:], (P, P), BF16, f"w2b{b}")
                      for b in ("td", "bu")}
            dinvp_sb = {b: _load(nc, cst, ins[b]["dinv_part"][:], (P, TBL), F32,
                                 f"dinvp{b}") for b in ("td", "bu")}

            import contextlib
            loop_ctx = tc.For_i(0, reps, 1) if reps > 1 else contextlib.nullcontext()
            with loop_ctx:
                _build_l2_body(
                    nc, pool, cst, psum, pool_ps, pp, ins, mlp_w1,
                    mlp_b1, mlp_w2, mlp_b2, out, iota_sb, ident_sb, ones_sb,
                    GL_sb, cinv_sb, rx0_sb, w2a_sb, w2b_sb, dinvp_sb, NCH)
    nc.compile()
    return nc


def _build_l2_body(nc, pool, cst, psum, pool_ps, pp, ins, mlp_w1,
                   mlp_b1, mlp_w2, mlp_b2, out, iota_sb, ident_sb, ones_sb,
                   GL_sb, cinv_sb, rx0_sb, w2a_sb, w2b_sb, dinvp_sb, NCH):
    TBL, T = pp["TBL"], pp["meta"]["T"]
    G_LOC, B_PAD = pp["meta"]["G_LOC"], pp["B_PAD"]
    br = pp["branches"]
    # ---- R_b = relu(x0[roots]) @ W2b_b, SBUF-resident [128g x 128f] x NCH ----
    R_sb = {}
    for b in ("td", "bu"):
        R_sb[b] = cst.tile([P, NCH * P], BF16, tag=f"R{b}", name=f"R{b}")
        for j in range(NCH):
            psr = psum.tile([P, P], F32, tag="ps")
            nc.tensor.matmul(psr[:], lhsT=rx0_sb[:, j * P: (j + 1) * P],
                             rhs=w2b_sb[b][:], start=True, stop=True)
            nc.scalar.activation(out=R_sb[b][:, j * P: (j + 1) * P], in_=psr[:],
                                 func=mybir.ActivationFunctionType.Copy)

    # ---- ht2 tables: dinv_b * (relu(x2_b) @ W2a_b), bf16 in DRAM ----
    CH = 8
    for b in ("td", "bu"):
        ib = ins[b]
        for c0 in range(0, TBL, CH):
            nb = min(CH, TBL - c0)
            xt = pool.tile([P, CH * P], BF16, tag="xt", bufs=2)
            nc.sync.dma_start(out=xt[:, : nb * P],
                              in_=ib["x2rT"][:, c0 * P: (c0 + nb) * P])
            st = pool.tile([P, CH * P], BF16, tag="st", bufs=2)
            for j in range(nb):
                blk = c0 + j
                psx = psum.tile([P, P], F32, tag="ps")
                nc.tensor.matmul(psx[:], lhsT=xt[:, j * P: (j + 1) * P],
                                 rhs=w2a_sb[b][:], start=True, stop=True)
                nc.scalar.activation(
                    out=st[:, j * P: (j + 1) * P], in_=psx[:],
                    func=mybir.ActivationFunctionType.Copy,
                    scale=dinvp_sb[b][:, blk: blk + 1])
            nc.sync.dma_start(
                out=ib["table"][c0 * P: (c0 + nb) * P, :]
                .rearrange("(j p) f -> p j f", p=P),
                in_=st[:, : nb * P].rearrange("p (j f) -> p j f", f=P))

    # ---- aggregation + relu + pooling per branch ----
    pooled = {}
    for b in ("td", "bu"):
        ib = ins[b]
        M = br[b]["Mbar"]
        IDX_sb = _load(nc, pool, ib["IDX"][:], (P, M * 8), I16, "idx")
        DSTL_sb = _load(nc, pool, ib["DSTL"][:], (P, M), BF16, "dstl")
        dinvl_sb = _load(nc, cst, ib["dinv_loc"][:], (P, T), F32, f"dinvl{b}")
        sqdeg_sb = _load(nc, cst, ib["sqdeg"][:], (1, T * P), BF16, f"sqdeg{b}")
        b2_sb = _load(nc, cst, ib["b2"][:], (1, P), BF16, f"b2{b}")
        ps_pool = pool_ps.tile([G_LOC, P], F32, tag=f"pool{b}")
        t_last = max(t for t in range(T) if br[b]["mbar"][t] > 0)

        def pre(t, ps, ib=ib, sqdeg_sb=sqdeg_sb, b2_sb=b2_sb, Rb=R_sb[b]):
            nc.tensor.matmul(ps[:], lhsT=sqdeg_sb[0:1, t * P: (t + 1) * P],
                             rhs=b2_sb[0:1, :], start=True, stop=False)
            ct = pool.tile([P, NCH * P], BF16, tag="ct", bufs=3)
            nc.sync.dma_start(
                out=ct[:].rearrange("g (c d) -> g c d", d=P),
                in_=ib["C"][:, t * P: (t + 1) * P]
                .rearrange("(c g) d -> g c d", g=P))
            for ci in range(NCH):
                nc.tensor.matmul(ps[:], lhsT=ct[:, ci * P: (ci + 1) * P],
                                 rhs=Rb[:, ci * P: (ci + 1) * P],
                                 start=False, stop=False)

        def consume(t, ps, dinvl_sb=dinvl_sb, ps_pool=ps_pool, t_last=t_last):
            h2 = pool.tile([P, P], BF16, tag="h2", bufs=3)
            nc.scalar.activation(out=h2[:], in_=ps[:],
                                 func=mybir.ActivationFunctionType.Relu,
                                 scale=dinvl_sb[:, t: t + 1])
            oh = pool.tile([P, G_LOC], BF16, tag="oh", bufs=3)
            nc.vector.tensor_tensor(
                out=oh[:], in0=GL_sb[:, t: t + 1].to_broadcast([P, G_LOC]),
                in1=iota_sb[:, :G_LOC], op=mybir.AluOpType.is_equal)
            nc.tensor.matmul(ps_pool[:], lhsT=oh[:], rhs=h2[:],
                             start=(t == 0), stop=(t == t_last))

        _aggregate(nc, pool, psum, ib["table"], IDX_sb, DSTL_sb,
                   br[b], TBL * P, iota_sb, T, pre, 1 + NCH, consume)

        meanS = pool.tile([G_LOC, P], F32, tag=f"mean{b}", bufs=1)
        nc.vector.tensor_scalar(
            out=meanS[:], in0=ps_pool[:], scalar1=cinv_sb[:G_LOC, :],
            scalar2=None, op0=mybir.AluOpType.mult)
        pst = psum.tile([P, G_LOC], F32, tag="ps")
        nc.tensor.transpose(out=pst[:], in_=meanS[:],
                            identity=ident_sb[:G_LOC, :G_LOC])
        meanT = pool.tile([P, G_LOC], F32, tag=f"meanT{b}", bufs=1)
        nc.vector.tensor_copy(out=meanT[:], in_=pst[:])
        pooled[b] = meanT

    # ---- final MLP over this core's G_LOC graph slots ----
    rx2_sb = {b: _load(nc, cst, ins[b]["rootx2T"][:, :P], (P, P), F32,
                       f"rx2{b}") for b in ("td", "bu")}
    w1c_sb = cst.tile([P, 4 * 2 * P], F32, tag="mw1", name="mw1")
    nc.sync.dma_start(
        out=w1c_sb[:].rearrange("p (c o) -> p c o", c=4),
        in_=mlp_w1[:].rearrange("(c p) o -> p c o", p=P))
    b1m_sb = _load(nc, cst, mlp_b1[:], (1, 2 * P), F32, "mb1")
    w2c_sb = cst.tile([P, 2 * 2], F32, tag="mw2", name="mw2")
    nc.sync.dma_start(
        out=w2c_sb[:].rearrange("p (c o) -> p c o", c=2),
        in_=mlp_w2[:].rearrange("(c p) o -> p c o", p=P))
    b2m_sb = _load(nc, cst, mlp_b2[:], (1, 2), F32, "mb2")

    ps1 = psum.tile([G_LOC, 2 * P], F32, tag="ps")
    chunks = [pooled["bu"][:, :G_LOC], rx2_sb["bu"][:, :G_LOC],
              pooled["td"][:, :G_LOC], rx2_sb["td"][:, :G_LOC]]
    for ci, lhsT in enumerate(chunks):
        nc.tensor.matmul(ps1[:], lhsT=lhsT,
                         rhs=w1c_sb[:, ci * 2 * P: (ci + 1) * 2 * P],
                         start=(ci == 0), stop=False)
    nc.tensor.matmul(ps1[:], lhsT=ones_sb[0:1, :G_LOC], rhs=b1m_sb[0:1, :],
                     start=False, stop=True)
    h1 = pool.tile([G_LOC, 2 * P], F32, tag="mlph", bufs=1)
    nc.scalar.activation(out=h1[:], in_=ps1[:],
                         func=mybir.ActivationFunctionType.Relu)
    hT = []
    for ci in range(2):
        pst2 = psum.tile([P, G_LOC], F32, tag="ps")
        nc.tensor.transpose(out=pst2[:], in_=h1[:, ci * P: (ci + 1) * P],
                            identity=ident_sb[:G_LOC, :G_LOC])
        ht_sb = pool.tile([P, G_LOC], F32, tag=f"hT{ci}", bufs=1)
        nc.vector.tensor_copy(out=ht_sb[:], in_=pst2[:])
        hT.append(ht_sb)
    ps2 = psum.tile([G_LOC, 2], F32, tag="ps")
    for ci in range(2):
        nc.tensor.matmul(ps2[:], lhsT=hT[ci][:, :G_LOC],
                         rhs=w2c_sb[:, ci * 2: (ci + 1) * 2],
                         start=(ci == 0), stop=False)
    nc.tensor.matmul(ps2[:], lhsT=ones_sb[0:1, :G_LOC], rhs=b2m_sb[0:1, :],
                     start=False, stop=True)
    oo = pool.tile([G_LOC, 2], F32, tag="oo", bufs=1)
    nc.vector.tensor_copy(out=oo[:], in_=ps2[:])
    nc.sync.dma_start(out=out[:G_LOC, :], in_=oo[:])


# ----------------------------------------------------------------------------
# in_map assembly + kernel entry
# ----------------------------------------------------------------------------

def l1_in_maps(pp, w):
    br = pp["branches"]
    w12 = np.concatenate([w["td_w1"], w["bu_w1"]], axis=1).astype(BF)
    maps = []
    for c in range(N_CORES):
        m = {"x0T": pp["x0T"], "iota": pp["iota"],
             "w12": np.ascontiguousarray(w12)}
        for b in ("td", "bu"):
            bb = br[b]
            m[f"b1{b}"] = w[f"{b}_b1"].astype(BF).reshape(1, P)
            m[f"dinvp{b}"] = bb["dinv_part"]
            m[f"dinvl{b}"] = bb["dinv_loc"][c]
            m[f"sqdeg{b}"] = bb["sqdeg_loc"][c]
            m[f"IDX{b}"] = bb["packed"][c]["IDX16"]
            m[f"DSTL{b}"] = bb["packed"][c]["DSTL"]
        maps.append(m)
    return maps


def l2_in_maps(pp, w, x2rT, rootx2T):
    br = pp["branches"]
    ones = np.ones((1, P), np.float32)
    maps = []
    for c in range(N_CORES):
        m = {"iota": pp["iota"], "ident": pp["ident"],
             "GL": pp["GL"][c],
             "cinv": pp["cinv"][c], "rootx0T": pp["rootx0T"], "ones_row": ones,
             "mlp_w1": w["mlp_w1"].astype(np.float32),
             "mlp_b1": w["mlp_b1"].astype(np.float32).reshape(1, -1),
             "mlp_w2": w["mlp_w2"].astype(np.float32),
             "mlp_b2": w["mlp_b2"].astype(np.float32).reshape(1, -1)}
        for b in ("td", "bu"):
            bb = br[b]
            m[f"x2rT{b}"] = x2rT[b]
            m[f"w2a{b}"] = np.ascontiguousarray(w[f"{b}_w2"][:P].astype(BF))
            m[f"w2b{b}"] = np.ascontiguousarray(w[f"{b}_w2"][P:].astype(BF))
            m[f"b2{b}"] = w[f"{b}_b2"].astype(BF).reshape(1, P)
            m[f"dinvp{b}"] = bb["dinv_part"]
            m[f"dinvl{b}"] = bb["dinv_loc"][c]
            m[f"sqdeg{b}"] = bb["sqdeg_loc"][c]
            m[f"C{b}"] = bb["C"][c]
            m[f"IDX{b}"] = bb["packed"][c]["IDX16"]
            m[f"DSTL{b}"] = bb["packed"][c]["DSTL"]
            m[f"rx2T{b}"] = rootx2T[b][c]
        maps.append(m)
    return maps


def assemble_x2(pp, results, b):
    N, meta = pp["N"], pp["meta"]
    ns, g0 = meta["node_start"], meta["g0"]
    x2 = np.zeros((N, P), np.float32)
    for c in range(N_CORES):
        lo, hi = int(ns[g0[c]]), int(ns[g0[c + 1]])
        x2[lo:hi] = results[c][f"x2{b}"][: hi - lo]
    return x2


def make_x2rT(pp, x2):
    """[128, TBL*128] bf16 relu(x2) transposed — layer-2 table-build input."""
    N, TBL = pp["N"], pp["TBL"]
    xt = np.zeros((P, TBL * P), BF)
    xt[:, :N] = np.maximum(x2, 0.0).T.astype(BF)
    return xt


def make_rootx2T(pp, x2, rootindex):
    """Per-core [128, 128] (padded from G_LOC) transposed root features."""
    meta = pp["meta"]
    G_LOC = meta["G_LOC"]
    root = np.asarray(rootindex).astype(np.int64)
    percore = []
    for c in range(N_CORES):
        gi = np.minimum(meta["g0"][c] + np.arange(G_LOC), pp["B"] - 1)
        rt = np.zeros((P, P), np.float32)
        rt[:, :G_LOC] = x2[root[gi]].T
        percore.append(rt)
    return percore


def _run(nc, in_maps):
    return run_bass_kernel_spmd(nc, in_maps, core_ids=list(range(N_CORES))).results


def kernel(x, x_da, edge_index, batch, rootindex,
           td_w1, td_b1, td_w2, td_b2,
           bu_w1, bu_b1, bu_w2, bu_b2,
           mlp_w1, mlp_b1, mlp_w2, mlp_b2):
    w = {"td_w1": td_w1, "td_b1": td_b1, "td_w2": td_w2, "td_b2": td_b2,
         "bu_w1": bu_w1, "bu_b1": bu_b1, "bu_w2": bu_w2, "bu_b2": bu_b2,
         "mlp_w1": mlp_w1, "mlp_b1": mlp_b1, "mlp_w2": mlp_w2, "mlp_b2": mlp_b2}
    w = {k: np.asarray(v) for k, v in w.items()}
    pp = preprocess(np.asarray(x), np.asarray(x_da), np.asarray(edge_index),
                    np.asarray(batch), np.asarray(rootindex))

    nc1 = build_l1(pp)
    res1 = _run(nc1, l1_in_maps(pp, w))

    x2rT, rootx2T = {}, {}
    for b in ("td", "bu"):
        x2 = assemble_x2(pp, res1, b)
        x2rT[b] = make_x2rT(pp, x2)
        rootx2T[b] = make_rootx2T(pp, x2, rootindex)

    nc2 = build_l2(pp)
    res2 = _run(nc2, l2_in_maps(pp, w, x2rT, rootx2T))

    B = pp["B"]
    meta = pp["meta"]
    out = np.zeros((B, 2), np.float32)
    for c in range(N_CORES):
        g0, g1 = meta["g0"][c], meta["g0"][c + 1]
        out[g0:g1] = res2[c]["out"][: g1 - g0]
    return out


# revision 18
# speedup vs baseline: 12.1217x; 1.4868x over previous
"""BiGCN (two-branch GCN + root-extend + scatter-mean + MLP) on 8 trn2 NeuronCores.

Sharding: nodes/edges are sharded by destination across 8 cores using
graph-aligned windows (so scatter-mean pooling stays core-local); the small
weight matrices are replicated. Two SPMD launches (layer-1, then
layer-2+pool+MLP) with host reassembly of layer-1 activations in between.

Per conv layer on device: build the full normalized feature table
ht = dinv * (act @ W) in DRAM (node-major, bf16), then per 128-dst-node tile:
indirect-DMA gather of ht[src] messages (bf16, queue-rotated across the 4
SWDGE queues), one-hot A01 = (dstlocal == iota) built on the vector engine,
PE bf16 matmul segment-sum into PSUM. The GCN bias is folded in as an extra
rank-1 "edge" (outer(sqrt(deg), b)) so the PSUM evacuation is a single
scalar-engine activation (scale by dinv, optional relu). Layer 2's
root-extend term  sum_e dinv[s]*R[batch[s]]  is precomputed on the host as a
dense [B_PAD, NLOC] matrix C and folded into the same PSUM accumulation via
4 extra matmuls per tile against the SBUF-resident R = relu(x0[root]) @ W2b.
"""
import os

import numpy as np

import concourse.bacc as bacc
import concourse.mybir as mybir
import concourse.tile as tile
from concourse.bass_utils import run_bass_kernel_spmd

P = 128
N_CORES = 8
F32 = mybir.dt.float32
BF16 = mybir.dt.bfloat16
BF = mybir.dt.np(mybir.dt.bfloat16)
I16 = mybir.dt.int16


# ----------------------------------------------------------------------------
# host-side preprocessing (index manipulation only)
# ----------------------------------------------------------------------------

def _ceil(a, b):
    return -(-a // b)


def _shard_meta(batch, B, N):
    """Graph-aligned per-core node windows, padded to a uniform 128-aligned
    size. Core c owns graphs [g0[c], g0[c+1]); it computes a window of NLOC
    nodes starting at its first owned node (covering all owned graphs plus a
    partial tail that is discarded)."""
    node_start = np.searchsorted(batch, np.arange(B + 1))
    g0 = [int(_ceil(B * c, N_CORES)) for c in range(N_CORES + 1)]
    spans = [int(node_start[g0[c + 1]] - node_start[g0[c]]) for c in range(N_CORES)]
    NLOC = _ceil(max(spans), P) * P
    T = NLOC // P
    n0 = [int(node_start[g0[c]]) for c in range(N_CORES)]
    gcounts = []
    for c in range(N_CORES):
        hi = min(n0[c] + NLOC, N)
        glast = int(batch[hi - 1]) if hi > n0[c] else g0[c]
        gcounts.append(glast - g0[c] + 1)
    G_LOC = max(gcounts)
    assert G_LOC <= P, f"G_LOC={G_LOC} exceeds 128 partitions"
    return {"node_start": node_start, "g0": g0, "n0": n0, "NLOC": NLOC,
            "T": T, "G_LOC": G_LOC}


CHK = 32768  # dma_gather table-chunk rows (int16 index range)


def _edges_for_core(src, dst, n0, NLOC, N, T, NQ):
    """Edges with dst in this core's window PLUS one self-edge per real
    window node (the GCN self-loop has exactly the edge normalization
    dinv_d*dinv_d, so it is just an extra (d, d) edge). Sorted by
    (dst tile, src); per-(tile, src-chunk) counts."""
    lo, hi = n0, min(n0 + NLOC, N)
    m = (dst >= lo) & (dst < hi)
    es = src[m].astype(np.int64)
    ed = (dst[m] - lo).astype(np.int64)
    sl = np.arange(lo, hi, dtype=np.int64)
    es = np.concatenate([es, sl])
    ed = np.concatenate([ed, sl - lo])
    tl = ed >> 7
    order = np.lexsort((es, tl))
    es, ed, tl = es[order], ed[order], tl[order]
    q = es >> 15
    cnt_tq = np.bincount(tl * NQ + q, minlength=T * NQ).reshape(T, NQ)
    return es, ed, tl, q, cnt_tq


TG = int(os.environ.get("K_TG", "1"))  # dst tiles per merged gather group
# TG>1 merges gather calls across dst tiles; >=2 has crashed HW (SWDGE ring
# carveout overflow for ~3k+ descriptor calls) — keep 1 unless re-validated.


def _pack_edges(branch_cores, T, NQ):
    """Union-max per-(tile, chunk) block counts sb[t][q]. Blocks are laid out
    group-major — for each group of TG tiles: for each chunk q: tiles in
    order — so one dma_gather covers a whole (group, chunk) run. Per-core
    padded arrays: IDX16 [128, Mbar*8] int16 (dma_gather wrapped layout, idx
    relative to chunk, pad=0) and DSTL [128, Mbar] bf16 (pad=-1). Flat edge
    slot j of segment (t,q) at block boff[t][q]+j//128, partition j%128 —
    exactly dma_gather's output layout."""
    sb = np.stack([(c["cnt_tq"] + P - 1) // P for c in branch_cores]).max(axis=0)
    ngroups = _ceil(T, TG)
    boff = np.zeros((T, NQ), int)
    goff = np.zeros(ngroups + 1, int)
    gb = np.zeros((ngroups, NQ), int)
    cum = 0
    for g in range(ngroups):
        goff[g] = cum
        for q in range(NQ):
            for t in range(g * TG, min((g + 1) * TG, T)):
                boff[t][q] = cum
                cum += sb[t][q]
                gb[g][q] += sb[t][q]
    goff[ngroups] = cum
    mb = sb.sum(axis=1)
    Mbar = max(1, int(cum))
    out = []
    for c in branch_cores:
        F = np.zeros(Mbar * P, np.int16)
        DSTL = np.full((P, Mbar), -1.0, BF)
        es, ed, tl, q, cnt_tq = (c["es"], c["ed"], c["tl"], c["q"], c["cnt_tq"])
        if len(ed):
            segid = tl * NQ + q
            starts = np.concatenate([[0], np.cumsum(cnt_tq.ravel())])
            within = np.arange(len(ed)) - starts[segid]
            flat = boff.ravel()[segid] * P + within
            F[flat] = (es & (CHK - 1)).astype(np.int16)
            DSTL[flat & 127, flat >> 7] = (ed - (tl << 7)).astype(np.float32)
        IDX16 = np.ascontiguousarray(np.tile(F.reshape(-1, 16).T, (8, 1)))
        out.append({"IDX16": IDX16, "DSTL": DSTL})
    return (sb.astype(int), boff.astype(int), mb.astype(int).tolist(),
            goff.astype(int), gb.astype(int), Mbar, out)


def _part_major(vec, TB, fill):
    v = np.full(TB * P, fill, vec.dtype)
    v[: len(vec)] = vec
    return np.ascontiguousarray(v.reshape(TB, P).T)


def preprocess(x, x_da, edge_index, batch, rootindex):
    import scipy.sparse as sp
    N = x.shape[0]
    B = rootindex.shape[0]
    x0 = np.concatenate([x, x_da], axis=1).astype(np.float32)
    assert x0.shape[1] == P
    TBL = _ceil(N, P)
    x0T = np.zeros((P, TBL * P), BF)
    x0T[:, :N] = x0.T.astype(BF)
    batch = batch.astype(np.int64)
    meta = _shard_meta(batch, B, N)
    T, NLOC, G_LOC = meta["T"], meta["NLOC"], meta["G_LOC"]
    B_PAD = _ceil(B, P) * P

    src_g = edge_index[0].astype(np.int64)
    dst_g = edge_index[1].astype(np.int64)

    NQ = _ceil(TBL * P, CHK)
    branches = {}
    for name, (s, d) in {"td": (src_g, dst_g), "bu": (dst_g, src_g)}.items():
        deg = (np.bincount(d, minlength=N) + 1.0).astype(np.float64)
        dinv = (1.0 / np.sqrt(deg)).astype(np.float32)
        sqdeg = np.sqrt(deg).astype(np.float32)
        cores = []
        for c in range(N_CORES):
            es, ed, tl, q, cnt_tq = _edges_for_core(
                s, d, meta["n0"][c], NLOC, N, T, NQ)
            cores.append({"es": es, "ed": ed, "tl": tl, "q": q,
                          "cnt_tq": cnt_tq})
        sb, boff, mb, goff, gb, Mbar, packed = _pack_edges(cores, T, NQ)
        bd = {"dinv": dinv, "sb": sb, "boff": boff, "mbar": mb,
              "goff": goff, "gb": gb, "Mbar": Mbar, "packed": packed,
              "dinv_part": _part_major(dinv, TBL, np.float32(1.0))}
        loc_dinv, loc_sqdeg, loc_C = [], [], []
        for c in range(N_CORES):
            rows = meta["n0"][c] + np.arange(NLOC)
            valid = rows < N
            rr = np.minimum(rows, N - 1)
            dv = np.where(valid, dinv[rr], 1.0).astype(np.float32)
            loc_dinv.append(np.ascontiguousarray(dv.reshape(T, P).T))
            sq = np.where(valid, sqdeg[rr], 0.0).astype(BF)
            loc_sqdeg.append(np.ascontiguousarray(sq.reshape(1, NLOC)))
            # C[g, dloc] = sum_{e: s->d, d local} dinv[s] * [batch[s] == g]
            # (+ self-loop dinv[d] at batch[d]) — layer-2 root-extend operand.
            ec = cores[c]
            gsrc = batch[ec["es"]]
            w = dinv[ec["es"]]
            Cm = sp.coo_matrix((w, (gsrc, ec["ed"])), shape=(B_PAD, NLOC)).toarray()
            loc_C.append(np.ascontiguousarray(Cm.astype(BF)))
        bd["dinv_loc"] = loc_dinv
        bd["sqdeg_loc"] = loc_sqdeg
        bd["C"] = loc_C
        branches[name] = bd

    GL, cinv = [], []
    for c in range(N_CORES):
        rows = meta["n0"][c] + np.arange(NLOC)
        valid = rows < N
        rr = np.minimum(rows, N - 1)
        gl = np.where(valid, batch[rr] - meta["g0"][c], -1).astype(np.float32)
        gl = np.where(gl < G_LOC, gl, -1.0).astype(np.float32)
        GL.append(np.ascontiguousarray(gl.reshape(T, P).T.astype(BF)))
        cnts = np.ones(P, np.float32)
        ns = meta["node_start"]
        for j in range(G_LOC):
            g = meta["g0"][c] + j
            if g < B:
                cc = float(ns[g + 1] - ns[g])
                cnts[j] = cc if cc > 0 else 1.0
        cinv.append((1.0 / cnts).reshape(P, 1).astype(np.float32))

    rootx0T = np.zeros((P, B_PAD), BF)
    rootx0T[:, :B] = np.maximum(x0[rootindex.astype(np.int64)], 0.0).T.astype(BF)

    iota = np.broadcast_to(np.arange(P, dtype=np.float32), (P, P)).astype(BF).copy()
    ident = np.eye(P, dtype=np.float32)

    return {"N": N, "B": B, "TBL": TBL, "B_PAD": B_PAD, "NQ": NQ,
            "meta": meta, "x0": x0, "x0T": x0T, "branches": branches,
            "GL": GL, "cinv": cinv, "rootx0T": rootx0T,
            "iota": iota, "ident": ident}


# ----------------------------------------------------------------------------
# device program builders
# ----------------------------------------------------------------------------

import os
N_QUEUES = int(os.environ.get("K_QUEUES", "4"))
K_SP = os.environ.get("K_SP", "1") == "1"   # single_packet for gathers
K_ROT = os.environ.get("K_ROT", "0") == "1"  # rotate gathers across queues (UNSAFE: sem-lane/queue ordering)
_qctr = [0]


def _next_q():
    if not K_ROT:
        return 0
    q = _qctr[0] % N_QUEUES
    _qctr[0] += 1
    return q


K_SCRATCH = int(os.environ.get("K_SCRATCH", "16384"))  # SWDGE ring carveout B/partition


def _new_nc():
    return bacc.Bacc("TRN2", target_bir_lowering=False, debug=False,
                     num_devices=N_CORES, num_swdge_queues=N_QUEUES,
                     dynamic_dma_scratch_size=K_SCRATCH)


def _load(nc, pool, dram_ap, shape, dtype, tag, bufs=1):
    t = pool.tile(list(shape), dtype, tag=tag, bufs=bufs)
    nc.sync.dma_start(out=t[:], in_=dram_ap)
    return t


def _aggregate(nc, pool, psum, table, IDX_sb, DSTL_sb, bmeta, TROWS,
               iota_sb, T, pre, npre, consume):
    """Group-merged segment-sum: per group of TG dst tiles, one dma_gather
    per 32k-row table chunk fetches all the group's messages, one vector op
    builds all its one-hot blocks, then per tile: PSUM_t = pre-matmuls +
    sum_e A01 . msg (self-loops are real edges) and consume(t, ps) finishes
    (scale/relu/pool/write)."""
    sb, boff, mbar = bmeta["sb"], bmeta["boff"], bmeta["mbar"]
    goff, gb = bmeta["goff"], bmeta["gb"]
    NQ = sb.shape[1]
    ngroups = len(gb)
    gsmax = max(1, int((goff[1:] - goff[:-1]).max()))
    for g in range(ngroups):
        gs = int(goff[g + 1] - goff[g])
        if gs == 0:
            continue  # window tail beyond N: no nodes, nothing to write
        msg = pool.tile([P, gsmax * P], BF16, tag="msg", bufs=3 if TG == 1 else 2)
        col = 0
        for q in range(NQ):
            nb = int(gb[g][q])
            if nb == 0:
                continue
            base = q * CHK
            rows = min(CHK, TROWS - base)
            b0 = int(goff[g]) + col
            nc.gpsimd.dma_gather(
                out_ap=msg[:, col * P: (col + nb) * P]
                .rearrange("p (b f) -> p b f", f=P),
                in_ap=table[base: base + rows, :],
                idxs_ap=IDX_sb[:, b0 * 8: (b0 + nb) * 8],
                num_idxs=nb * P, num_idxs_reg=nb * P, elem_size=P,
                queue_num=_next_q(), single_packet=K_SP)
            col += nb
        a01 = pool.tile([P, gsmax * P], BF16, tag="a01", bufs=3 if TG == 1 else 2)
        nc.vector.tensor_tensor(
            out=a01[:, : gs * P].rearrange("p (k f) -> p k f", f=P),
            in0=DSTL_sb[:, goff[g]: goff[g] + gs].to_broadcast([P, gs, P]),
            in1=iota_sb[:].unsqueeze(1).broadcast_to([P, gs, P]),
            op=mybir.AluOpType.is_equal,
        )
        for t in range(g * TG, min((g + 1) * TG, T)):
            if mbar[t] == 0:
                continue
            ps = psum.tile([P, P], F32, tag="ps")
            pre(t, ps)  # npre matmuls, first with start=True
            nk, klast = 0, mbar[t] - 1
            for q in range(NQ):
                for j in range(int(sb[t][q])):
                    k = int(boff[t][q]) - int(goff[g]) + j
                    nc.tensor.matmul(ps[:], lhsT=a01[:, k * P: (k + 1) * P],
                                     rhs=msg[:, k * P: (k + 1) * P],
                                     start=(npre == 0 and nk == 0),
                                     stop=(nk == klast))
                    nk += 1
            consume(t, ps)


def build_l1(pp, reps=1):
    _qctr[0] = 0
    TBL, T = pp["TBL"], pp["meta"]["T"]
    NLOC = pp["meta"]["NLOC"]
    br = pp["branches"]
    nc = _new_nc()
    x0T = nc.dram_tensor("x0T", [P, TBL * P], BF16, kind="ExternalInput")
    iota = nc.dram_tensor("iota", [P, P], BF16, kind="ExternalInput")
    w12 = nc.dram_tensor("w12", [P, 2 * P], BF16, kind="ExternalInput")
    ins = {}
    for b in ("td", "bu"):
        M = br[b]["Mbar"]
        ins[b] = {
            "b1": nc.dram_tensor(f"b1{b}", [1, P], BF16, kind="ExternalInput"),
            "dinv_part": nc.dram_tensor(f"dinvp{b}", [P, TBL], F32, kind="ExternalInput"),
            "dinv_loc": nc.dram_tensor(f"dinvl{b}", [P, T], F32, kind="ExternalInput"),
            "sqdeg": nc.dram_tensor(f"sqdeg{b}", [1, NLOC], BF16, kind="ExternalInput"),
            "IDX": nc.dram_tensor(f"IDX{b}", [P, M * 8], I16, kind="ExternalInput"),
            "DSTL": nc.dram_tensor(f"DSTL{b}", [P, M], BF16, kind="ExternalInput"),
            "table": nc.dram_tensor(f"table{b}", [TBL * P, P], BF16, kind="Internal"),
            "x2": nc.dram_tensor(f"x2{b}", [T * P, P], F32, kind="ExternalOutput"),
        }

    with tile.TileContext(nc) as tc:
        with (
            tc.tile_pool(name="sbuf", bufs=2) as pool,
            tc.tile_pool(name="cst", bufs=1) as cst,
            tc.tile_pool(name="psum", bufs=4, space="PSUM") as psum,
        ):
            iota_sb = _load(nc, cst, iota[:], (P, P), BF16, "iota")
            w12_sb = _load(nc, cst, w12[:], (P, 2 * P), BF16, "w12")
            dinvp_sb = {b: _load(nc, cst, ins[b]["dinv_part"][:], (P, TBL), F32,
                                 f"dinvp{b}") for b in ("td", "bu")}

            import contextlib
            loop_ctx = tc.For_i(0, reps, 1) if reps > 1 else contextlib.nullcontext()
            with loop_ctx:
                _build_l1_body(nc, x0T, pool, cst, psum, pp, ins, iota_sb,
                               w12_sb, dinvp_sb)
    nc.compile()
    return nc


def _build_l1_body(nc, x0T, pool, cst, psum, pp, ins, iota_sb, w12_sb,
                   dinvp_sb):
    TBL, T = pp["TBL"], pp["meta"]["T"]
    br = pp["branches"]
    # ---- tables: ht_b = dinv_b * (x0 @ W1_b), node-major bf16 in DRAM ----
    CH = 8
    for c0 in range(0, TBL, CH):
        nb = min(CH, TBL - c0)
        xt = pool.tile([P, CH * P], BF16, tag="xt", bufs=2)
        nc.sync.dma_start(out=xt[:, : nb * P],
                          in_=x0T[:, c0 * P: (c0 + nb) * P])
        st = {b: pool.tile([P, CH * P], BF16, tag=f"st{b}", bufs=2,
                           name=f"st{b}")
              for b in ("td", "bu")}
        for j in range(nb):
            blk = c0 + j
            psx = psum.tile([P, 2 * P], F32, tag="psx")
            nc.tensor.matmul(psx[:], lhsT=xt[:, j * P: (j + 1) * P],
                             rhs=w12_sb[:], start=True, stop=True)
            for bi, b in enumerate(("td", "bu")):
                nc.scalar.activation(
                    out=st[b][:, j * P: (j + 1) * P],
                    in_=psx[:, bi * P: (bi + 1) * P],
                    func=mybir.ActivationFunctionType.Copy,
                    scale=dinvp_sb[b][:, blk: blk + 1])
        for b in ("td", "bu"):
            nc.sync.dma_start(
                out=ins[b]["table"][c0 * P: (c0 + nb) * P, :]
                .rearrange("(j p) f -> p j f", p=P),
                in_=st[b][:, : nb * P].rearrange("p (j f) -> p j f", f=P))

    # ---- aggregation per branch ----
    for b in ("td", "bu"):
        ib = ins[b]
        M = br[b]["Mbar"]
        IDX_sb = _load(nc, pool, ib["IDX"][:], (P, M * 8), I16, "idx")
        DSTL_sb = _load(nc, pool, ib["DSTL"][:], (P, M), BF16, "dstl")
        dinvl_sb = _load(nc, cst, ib["dinv_loc"][:], (P, T), F32, f"dinvl{b}")
        sqdeg_sb = _load(nc, cst, ib["sqdeg"][:], (1, T * P), BF16, f"sqdeg{b}")
        b1_sb = _load(nc, cst, ib["b1"][:], (1, P), BF16, f"b1{b}")

        def pre(t, ps, sqdeg_sb=sqdeg_sb, b1_sb=b1_sb):
            nc.tensor.matmul(ps[:], lhsT=sqdeg_sb[0:1, t * P: (t + 1) * P],
                             rhs=b1_sb[0:1, :], start=True, stop=False)

        def consume(t, ps, ib=ib, dinvl_sb=dinvl_sb):
            xo = pool.tile([P, P], F32, tag="xo", bufs=3)
            nc.scalar.activation(out=xo[:], in_=ps[:],
                                 func=mybir.ActivationFunctionType.Copy,
                                 scale=dinvl_sb[:, t: t + 1])
            nc.sync.dma_start(out=ib["x2"][t * P: (t + 1) * P, :], in_=xo[:])

        _aggregate(nc, pool, psum, ib["table"], IDX_sb, DSTL_sb,
                   br[b], TBL * P, iota_sb, T, pre, 1, consume)


def build_l2(pp, reps=1):
    _qctr[0] = 0
    TBL, T = pp["TBL"], pp["meta"]["T"]
    G_LOC, B_PAD = pp["meta"]["G_LOC"], pp["B_PAD"]
    NLOC = pp["meta"]["NLOC"]
    br = pp["branches"]
    nc = _new_nc()
    iota = nc.dram_tensor("iota", [P, P], BF16, kind="ExternalInput")
    ident = nc.dram_tensor("ident", [P, P], F32, kind="ExternalInput")
    GL = nc.dram_tensor("GL", [P, T], BF16, kind="ExternalInput")
    cinv = nc.dram_tensor("cinv", [P, 1], F32, kind="ExternalInput")
    rootx0T = nc.dram_tensor("rootx0T", [P, B_PAD], BF16, kind="ExternalInput")
    ones_row = nc.dram_tensor("ones_row", [1, P], F32, kind="ExternalInput")
    mlp_w1 = nc.dram_tensor("mlp_w1", [4 * P, 2 * P], F32, kind="ExternalInput")
    mlp_b1 = nc.dram_tensor("mlp_b1", [1, 2 * P], F32, kind="ExternalInput")
    mlp_w2 = nc.dram_tensor("mlp_w2", [2 * P, 2], F32, kind="ExternalInput")
    mlp_b2 = nc.dram_tensor("mlp_b2", [1, 2], F32, kind="ExternalInput")
    out = nc.dram_tensor("out", [P, 2], F32, kind="ExternalOutput")
    NCH = B_PAD // P  # 4 root/graph chunks
    ins = {}
    for b in ("td", "bu"):
        M = br[b]["Mbar"]
        ins[b] = {
            "x2rT": nc.dram_tensor(f"x2rT{b}", [P, TBL * P], BF16, kind="ExternalInput"),
            "w2a": nc.dram_tensor(f"w2a{b}", [P, P], BF16, kind="ExternalInput"),
            "w2b": nc.dram_tensor(f"w2b{b}", [P, P], BF16, kind="ExternalInput"),
            "b2": nc.dram_tensor(f"b2{b}", [1, P], BF16, kind="ExternalInput"),
            "dinv_part": nc.dram_tensor(f"dinvp{b}", [P, TBL], F32, kind="ExternalInput"),
            "dinv_loc": nc.dram_tensor(f"dinvl{b}", [P, T], F32, kind="ExternalInput"),
            "sqdeg": nc.dram_tensor(f"sqdeg{b}", [1, NLOC], BF16, kind="ExternalInput"),
            "C": nc.dram_tensor(f"C{b}", [B_PAD, NLOC], BF16, kind="ExternalInput"),
            "IDX": nc.dram_tensor(f"IDX{b}", [P, M * 8], I16, kind="ExternalInput"),
            "DSTL": nc.dram_tensor(f"DSTL{b}", [P, M], BF16, kind="ExternalInput"),
            "rootx2T": nc.dram_tensor(f"rx2T{b}", [P, P], F32, kind="ExternalInput"),
            "table": nc.dram_tensor(f"table{b}", [TBL * P, P], BF16, kind="Internal"),
        }

    with tile.TileContext(nc) as tc:
        with (
            tc.tile_pool(name="sbuf", bufs=2) as pool,
            tc.tile_pool(name="cst", bufs=1) as cst,
            tc.tile_pool(name="psum", bufs=4, space="PSUM") as psum,
            tc.tile_pool(name="pps", bufs=1, space="PSUM") as pool_ps,
        ):
            iota_sb = _load(nc, cst, iota[:], (P, P), BF16, "iota")
            ident_sb = _load(nc, cst, ident[:], (P, P), F32, "ident")
            ones_sb = _load(nc, cst, ones_row[:], (1, P), F32, "ones")
            GL_sb = _load(nc, cst, GL[:], (P, T), BF16, "GL")
            cinv_sb = _load(nc, cst, cinv[:], (P, 1), F32, "cinv")
            rx0_sb = _load(nc, cst, rootx0T[:], (P, B_PAD), BF16, "rx0")
            w2a_sb = {b: _load(nc, cst, ins[b]["w2a"][:], (P, P), BF16, f"w2a{b}")
                      for b in ("td", "bu")}
            w2b_sb = {b: _load(nc, cst, ins[b]["w2b"][:], (P, P), BF16, f"w2b{b}")
                      for b in ("td", "bu")}
            dinvp_sb = {b: _load(nc, cst, ins[b]["dinv_part"][:], (P, TBL), F32,
                                 f"dinvp{b}") for b in ("td", "bu")}

            import contextlib
            loop_ctx = tc.For_i(0, reps, 1) if reps > 1 else contextlib.nullcontext()
            with loop_ctx:
                _build_l2_body(
                    nc, pool, cst, psum, pool_ps, pp, ins, mlp_w1,
                    mlp_b1, mlp_w2, mlp_b2, out, iota_sb, ident_sb, ones_sb,
                    GL_sb, cinv_sb, rx0_sb, w2a_sb, w2b_sb, dinvp_sb, NCH)
    nc.compile()
    return nc


def _build_l2_body(nc, pool, cst, psum, pool_ps, pp, ins, mlp_w1,
                   mlp_b1, mlp_w2, mlp_b2, out, iota_sb, ident_sb, ones_sb,
                   GL_sb, cinv_sb, rx0_sb, w2a_sb, w2b_sb, dinvp_sb, NCH):
    TBL, T = pp["TBL"], pp["meta"]["T"]
    G_LOC, B_PAD = pp["meta"]["G_LOC"], pp["B_PAD"]
    br = pp["branches"]
    # ---- R_b = relu(x0[roots]) @ W2b_b, SBUF-resident [128g x 128f] x NCH ----
    R_sb = {}
    for b in ("td", "bu"):
        R_sb[b] = cst.tile([P, NCH * P], BF16, tag=f"R{b}", name=f"R{b}")
        for j in range(NCH):
            psr = psum.tile([P, P], F32, tag="ps")
            nc.tensor.matmul(psr[:], lhsT=rx0_sb[:, j * P: (j + 1) * P],
                             rhs=w2b_sb[b][:], start=True, stop=True)
            nc.scalar.activation(out=R_sb[b][:, j * P: (j + 1) * P], in_=psr[:],
                                 func=mybir.ActivationFunctionType.Copy)

    # ---- ht2 tables: dinv_b * (relu(x2_b) @ W2a_b), bf16 in DRAM ----
    CH = 8
    for b in ("td", "bu"):
        ib = ins[b]
        for c0 in range(0, TBL, CH):
            nb = min(CH, TBL - c0)
            xt = pool.tile([P, CH * P], BF16, tag="xt", bufs=2)
            nc.sync.dma_start(out=xt[:, : nb * P],
                              in_=ib["x2rT"][:, c0 * P: (c0 + nb) * P])
            st = pool.tile([P, CH * P], BF16, tag="st", bufs=2)
            for j in range(nb):
                blk = c0 + j
                psx = psum.tile([P, P], F32, tag="ps")
                nc.tensor.matmul(psx[:], lhsT=xt[:, j * P: (j + 1) * P],
                                 rhs=w2a_sb[b][:], start=True, stop=True)
                nc.scalar.activation(
                    out=st[:, j * P: (j + 1) * P], in_=psx[:],
                    func=mybir.ActivationFunctionType.Copy,
                    scale=dinvp_sb[b][:, blk: blk + 1])
            nc.sync.dma_start(
                out=ib["table"][c0 * P: (c0 + nb) * P, :]
                .rearrange("(j p) f -> p j f", p=P),
                in_=st[:, : nb * P].rearrange("p (j f) -> p j f", f=P))

    # ---- aggregation + relu + pooling per branch ----
    pooled = {}
    for b in ("td", "bu"):
        ib = ins[b]
        M = br[b]["Mbar"]
        IDX_sb = _load(nc, pool, ib["IDX"][:], (P, M * 8), I16, "idx")
        DSTL_sb = _load(nc, pool, ib["DSTL"][:], (P, M), BF16, "dstl")
        dinvl_sb = _load(nc, cst, ib["dinv_loc"][:], (P, T), F32, f"dinvl{b}")
        sqdeg_sb = _load(nc, cst, ib["sqdeg"][:], (1, T * P), BF16, f"sqdeg{b}")
        b2_sb = _load(nc, cst, ib["b2"][:], (1, P), BF16, f"b2{b}")
        ps_pool = pool_ps.tile([G_LOC, P], F32, tag=f"pool{b}")
        t_last = max(t for t in range(T) if br[b]["mbar"][t] > 0)

        def pre(t, ps, ib=ib, sqdeg_sb=sqdeg_sb, b2_sb=b2_sb, Rb=R_sb[b]):
            nc.tensor.matmul(ps[:], lhsT=sqdeg_sb[0:1, t * P: (t + 1) * P],
                             rhs=b2_sb[0:1, :], start=True, stop=False)
            ct = pool.tile([P, NCH * P], BF16, tag="ct", bufs=3)
            nc.sync.dma_start(
                out=ct[:].rearrange("g (c d) -> g c d", d=P),
                in_=ib["C"][:, t * P: (t + 1) * P]
                .rearrange("(c g) d -> g c d", g=P))
            for ci in range(NCH):
                nc.tensor.matmul(ps[:], lhsT=ct[:, ci * P: (ci + 1) * P],
                                 rhs=Rb[:, ci * P: (ci + 1) * P],
                                 start=False, stop=False)

        def consume(t, ps, dinvl_sb=dinvl_sb, ps_pool=ps_pool, t_last=t_last):
            h2 = pool.tile([P, P], BF16, tag="h2", bufs=3)
            nc.scalar.activation(out=h2[:], in_=ps[:],
                                 func=mybir.ActivationFunctionType.Relu,
                                 scale=dinvl_sb[:, t: t + 1])
            oh = pool.tile([P, G_LOC], BF16, tag="oh", bufs=3)
            nc.vector.tensor_tensor(
                out=oh[:], in0=GL_sb[:, t: t + 1].to_broadcast([P, G_LOC]),
                in1=iota_sb[:, :G_LOC], op=mybir.AluOpType.is_equal)
            nc.tensor.matmul(ps_pool[:], lhsT=oh[:], rhs=h2[:],
                             start=(t == 0), stop=(t == t_last))

        _aggregate(nc, pool, psum, ib["table"], IDX_sb, DSTL_sb,
                   br[b], TBL * P, iota_sb, T, pre, 1 + NCH, consume)

        meanS = pool.tile([G_LOC, P], F32, tag=f"mean{b}", bufs=1)
        nc.vector.tensor_scalar(
            out=meanS[:], in0=ps_pool[:], scalar1=cinv_sb[:G_LOC, :],
            scalar2=None, op0=mybir.AluOpType.mult)
        pst = psum.tile([P, G_LOC], F32, tag="ps")
        nc.tensor.transpose(out=pst[:], in_=meanS[:],
                            identity=ident_sb[:G_LOC, :G_LOC])
        meanT = pool.tile([P, G_LOC], F32, tag=f"meanT{b}", bufs=1)
        nc.vector.tensor_copy(out=meanT[:], in_=pst[:])
        pooled[b] = meanT

    # ---- final MLP over this core's G_LOC graph slots ----
    rx2_sb = {b: _load(nc, cst, ins[b]["rootx2T"][:, :P], (P, P), F32,
                       f"rx2{b}") for b in ("td", "bu")}
    w1c_sb = cst.tile([P, 4 * 2 * P], F32, tag="mw1", name="mw1")
    nc.sync.dma_start(
        out=w1c_sb[:].rearrange("p (c o) -> p c o", c=4),
        in_=mlp_w1[:].rearrange("(c p) o -> p c o", p=P))
    b1m_sb = _load(nc, cst, mlp_b1[:], (1, 2 * P), F32, "mb1")
    w2c_sb = cst.tile([P, 2 * 2], F32, tag="mw2", name="mw2")
    nc.sync.dma_start(
        out=w2c_sb[:].rearrange("p (c o) -> p c o", c=2),
        in_=mlp_w2[:].rearrange("(c p) o -> p c o", p=P))
    b2m_sb = _load(nc, cst, mlp_b2[:], (1, 2), F32, "mb2")

    ps1 = psum.tile([G_LOC, 2 * P], F32, tag="ps")
    chunks = [pooled["bu"][:, :G_LOC], rx2_sb["bu"][:, :G_LOC],
              pooled["td"][:, :G_LOC], rx2_sb["td"][:, :G_LOC]]
    for ci, lhsT in enumerate(chunks):
        nc.tensor.matmul(ps1[:], lhsT=lhsT,
                         rhs=w1c_sb[:, ci * 2 * P: (ci + 1) * 2 * P],
                         start=(ci == 0), stop=False)
    nc.tensor.matmul(ps1[:], lhsT=ones_sb[0:1, :G_LOC], rhs=b1m_sb[0:1, :],
                     start=False, stop=True)
    h1 = pool.tile([G_LOC, 2 * P], F32, tag="mlph", bufs=1)
    nc.scalar.activation(out=h1[:], in_=ps1[:],
                         func=mybir.ActivationFunctionType.Relu)
    hT = []
    for ci in range(2):
        pst2 = psum.tile([P, G_LOC], F32, tag="ps")
        nc.tensor.transpose(out=pst2[:], in_=h1[:, ci * P: (ci + 1) * P],
                            identity=ident_sb[:G_LOC, :G_LOC])
        ht_sb = pool.tile([P, G_LOC], F32, tag=f"hT{ci}", bufs=1)
        nc.vector.tensor_copy(out=ht_sb[:], in_=pst2[:])
        hT.append(ht_sb)
    ps2 = psum.tile([G_LOC, 2], F32, tag="ps")
    for ci in range(2):
        nc.tensor.matmul(ps2[:], lhsT=hT[ci][:, :G_LOC],
                         rhs=w2c_sb[:, ci * 2: (ci + 1) * 2],
                         start=(ci == 0), stop=False)
    nc.tensor.matmul(ps2[:], lhsT=ones_sb[0:1, :G_LOC], rhs=b2m_sb[0:1, :],
                     start=False, stop=True)
    oo = pool.tile([G_LOC, 2], F32, tag="oo", bufs=1)
    nc.vector.tensor_copy(out=oo[:], in_=ps2[:])
    nc.sync.dma_start(out=out[:G_LOC, :], in_=oo[:])


# ----------------------------------------------------------------------------
# in_map assembly + kernel entry
# ----------------------------------------------------------------------------

def l1_in_maps(pp, w):
    br = pp["branches"]
    w12 = np.concatenate([w["td_w1"], w["bu_w1"]], axis=1).astype(BF)
    maps = []
    for c in range(N_CORES):
        m = {"x0T": pp["x0T"], "iota": pp["iota"],
             "w12": np.ascontiguousarray(w12)}
        for b in ("td", "bu"):
            bb = br[b]
            m[f"b1{b}"] = w[f"{b}_b1"].astype(BF).reshape(1, P)
            m[f"dinvp{b}"] = bb["dinv_part"]
            m[f"dinvl{b}"] = bb["dinv_loc"][c]
            m[f"sqdeg{b}"] = bb["sqdeg_loc"][c]
            m[f"IDX{b}"] = bb["packed"][c]["IDX16"]
            m[f"DSTL{b}"] = bb["packed"][c]["DSTL"]
        maps.append(m)
    return maps


def l2_in_maps(pp, w, x2rT, rootx2T):
    br = pp["branches"]
    ones = np.ones((1, P), np.float32)
    maps = []
    for c in range(N_CORES):
        m = {"iota": pp["iota"], "ident": pp["ident"],
             "GL": pp["GL"][c],
             "cinv": pp["cinv"][c], "rootx0T": pp["rootx0T"], "ones_row": ones,
             "mlp_w1": w["mlp_w1"].astype(np.float32),
             "mlp_b1": w["mlp_b1"].astype(np.float32).reshape(1, -1),
             "mlp_w2": w["mlp_w2"].astype(np.float32),
             "mlp_b2": w["mlp_b2"].astype(np.float32).reshape(1, -1)}
        for b in ("td", "bu"):
            bb = br[b]
            m[f"x2rT{b}"] = x2rT[b]
            m[f"w2a{b}"] = np.ascontiguousarray(w[f"{b}_w2"][:P].astype(BF))
            m[f"w2b{b}"] = np.ascontiguousarray(w[f"{b}_w2"][P:].astype(BF))
            m[f"b2{b}"] = w[f"{b}_b2"].astype(BF).reshape(1, P)
            m[f"dinvp{b}"] = bb["dinv_part"]
            m[f"dinvl{b}"] = bb["dinv_loc"][c]
            m[f"sqdeg{b}"] = bb["sqdeg_loc"][c]
            m[f"C{b}"] = bb["C"][c]
            m[f"IDX{b}"] = bb["packed"][c]["IDX16"]
            m[f"DSTL{b}"] = bb["packed"][c]["DSTL"]
            m[f"rx2T{b}"] = rootx2T[b][c]
        maps.append(m)
    return maps


def assemble_x2(pp, results, b):
    N, meta = pp["N"], pp["meta"]
    ns, g0 = meta["node_start"], meta["g0"]
    x2 = np.zeros((N, P), np.float32)
    for c in range(N_CORES):
        lo, hi = int(ns[g0[c]]), int(ns[g0[c + 1]])
        x2[lo:hi] = results[c][f"x2{b}"][: hi - lo]
    return x2


def make_x2rT(pp, x2):
    """[128, TBL*128] bf16 relu(x2) transposed — layer-2 table-build input."""
    N, TBL = pp["N"], pp["TBL"]
    xt = np.zeros((P, TBL * P), BF)
    xt[:, :N] = np.maximum(x2, 0.0).T.astype(BF)
    return xt


def make_rootx2T(pp, x2, rootindex):
    """Per-core [128, 128] (padded from G_LOC) transposed root features."""
    meta = pp["meta"]
    G_LOC = meta["G_LOC"]
    root = np.asarray(rootindex).astype(np.int64)
    percore = []
    for c in range(N_CORES):
        gi = np.minimum(meta["g0"][c] + np.arange(G_LOC), pp["B"] - 1)
        rt = np.zeros((P, P), np.float32)
        rt[:, :G_LOC] = x2[root[gi]].T
        percore.append(rt)
    return percore


def _run(nc, in_maps):
    return run_bass_kernel_spmd(nc, in_maps, core_ids=list(range(N_CORES))).results


def kernel(x, x_da, edge_index, batch, rootindex,
           td_w1, td_b1, td_w2, td_b2,
           bu_w1, bu_b1, bu_w2, bu_b2,
           mlp_w1, mlp_b1, mlp_w2, mlp_b2):
    w = {"td_w1": td_w1, "td_b1": td_b1, "td_w2": td_w2, "td_b2": td_b2,
         "bu_w1": bu_w1, "bu_b1": bu_b1, "bu_w2": bu_w2, "bu_b2": bu_b2,
         "mlp_w1": mlp_w1, "mlp_b1": mlp_b1, "mlp_w2": mlp_w2, "mlp_b2": mlp_b2}
    w = {k: np.asarray(v) for k, v in w.items()}
    pp = preprocess(np.asarray(x), np.asarray(x_da), np.asarray(edge_index),
                    np.asarray(batch), np.asarray(rootindex))

    nc1 = build_l1(pp)
    res1 = _run(nc1, l1_in_maps(pp, w))

    x2rT, rootx2T = {}, {}
    for b in ("td", "bu"):
        x2 = assemble_x2(pp, res1, b)
        x2rT[b] = make_x2rT(pp, x2)
        rootx2T[b] = make_rootx2T(pp, x2, rootindex)

    nc2 = build_l2(pp)
    res2 = _run(nc2, l2_in_maps(pp, w, x2rT, rootx2T))

    B = pp["B"]
    meta = pp["meta"]
    out = np.zeros((B, 2), np.float32)
    for c in range(N_CORES):
        g0, g1 = meta["g0"][c], meta["g0"][c + 1]
        out[g0:g1] = res2[c]["out"][: g1 - g0]
    return out


# revision 33
# speedup vs baseline: 13.3308x; 1.0997x over previous
"""BiGCN (two-branch GCN + root-extend + scatter-mean + MLP) on 8 trn2 NeuronCores.

Sharding: nodes/edges are sharded by destination across 8 cores using
graph-aligned windows (so scatter-mean pooling stays core-local); the small
weight matrices are replicated. Two SPMD launches (layer-1, then
layer-2+pool+MLP) with host reassembly of layer-1 activations in between.

Per conv layer on device: build the full normalized feature table
ht = dinv * (act @ W) in DRAM (node-major, bf16), then per 128-dst-node tile:
indirect-DMA gather of ht[src] messages (bf16, queue-rotated across the 4
SWDGE queues), one-hot A01 = (dstlocal == iota) built on the vector engine,
PE bf16 matmul segment-sum into PSUM. The GCN bias is folded in as an extra
rank-1 "edge" (outer(sqrt(deg), b)) so the PSUM evacuation is a single
scalar-engine activation (scale by dinv, optional relu). Layer 2's
root-extend term  sum_e dinv[s]*R[batch[s]]  is precomputed on the host as a
dense [B_PAD, NLOC] matrix C and folded into the same PSUM accumulation via
4 extra matmuls per tile against the SBUF-resident R = relu(x0[root]) @ W2b.
"""
import os

import numpy as np

import concourse.bacc as bacc
import concourse.mybir as mybir
import concourse.tile as tile
from concourse.bass_utils import run_bass_kernel_spmd

P = 128
N_CORES = 8
F32 = mybir.dt.float32
BF16 = mybir.dt.bfloat16
BF = mybir.dt.np(mybir.dt.bfloat16)
I16 = mybir.dt.int16


# ----------------------------------------------------------------------------
# host-side preprocessing (index manipulation only)
# ----------------------------------------------------------------------------

def _ceil(a, b):
    return -(-a // b)


def _shard_meta(batch, B, N):
    """Graph-aligned per-core node windows, padded to a uniform 128-aligned
    size. Core c owns graphs [g0[c], g0[c+1]); it computes a window of NLOC
    nodes starting at its first owned node (covering all owned graphs plus a
    partial tail that is discarded)."""
    node_start = np.searchsorted(batch, np.arange(B + 1))
    g0 = [int(_ceil(B * c, N_CORES)) for c in range(N_CORES + 1)]
    spans = [int(node_start[g0[c + 1]] - node_start[g0[c]]) for c in range(N_CORES)]
    NLOC = _ceil(max(spans), P) * P
    T = NLOC // P
    n0 = [int(node_start[g0[c]]) for c in range(N_CORES)]
    gcounts = []
    for c in range(N_CORES):
        hi = min(n0[c] + NLOC, N)
        glast = int(batch[hi - 1]) if hi > n0[c] else g0[c]
        gcounts.append(glast - g0[c] + 1)
    G_LOC = max(gcounts)
    assert G_LOC <= P, f"G_LOC={G_LOC} exceeds 128 partitions"
    return {"node_start": node_start, "g0": g0, "n0": n0, "NLOC": NLOC,
            "T": T, "G_LOC": G_LOC}


CHK = 32768  # dma_gather table-chunk rows (int16 index range)


def _edges_for_core(src, dst, n0, NLOC, N, T, NQ):
    """Edges with dst in this core's window PLUS one self-edge per real
    window node (the GCN self-loop has exactly the edge normalization
    dinv_d*dinv_d, so it is just an extra (d, d) edge). Sorted by
    (dst tile, src); per-(tile, src-chunk) counts."""
    lo, hi = n0, min(n0 + NLOC, N)
    m = (dst >= lo) & (dst < hi)
    es = src[m].astype(np.int64)
    ed = (dst[m] - lo).astype(np.int64)
    sl = np.arange(lo, hi, dtype=np.int64)
    es = np.concatenate([es, sl])
    ed = np.concatenate([ed, sl - lo])
    tl = ed >> 7
    order = np.lexsort((es, tl))
    es, ed, tl = es[order], ed[order], tl[order]
    q = es >> 15
    cnt_tq = np.bincount(tl * NQ + q, minlength=T * NQ).reshape(T, NQ)
    return es, ed, tl, q, cnt_tq


TG = int(os.environ.get("K_TG", "1"))  # dst tiles per merged gather group
# TG>1 merges gather calls across dst tiles; >=2 crashes HW with BOTH 16KB
# and 48KB ring carveouts (suspect Q7 idx alloc_scratch, not just ring
# space) — keep 1.
K_REG = os.environ.get("K_REG", "0") == "1"  # exact per-core gather counts
# K_REG: pad idx slots with -1 and pass each call's true per-core edge count
# via a Pool register (reg_load) so the ucode skips pad descriptors (~24% of
# the gather stream is union-max/block padding). Requires TG == 1 (pads are
# call-trailing only then). CRASHED HW on first trial (mesh desync) — keep
# off unless re-debugged (suspect reg_load/dma_gather scheduling on Pool).


def _pack_edges(branch_cores, T, NQ):
    """Union-max per-(tile, chunk) block counts sb[t][q]. Blocks are laid out
    group-major — for each group of TG tiles: for each chunk q: tiles in
    order — so one dma_gather covers a whole (group, chunk) run. Per-core
    padded arrays: IDX16 [128, Mbar*8] int16 (dma_gather wrapped layout, idx
    relative to chunk, pad=0) and DSTL [128, Mbar] bf16 (pad=-1). Flat edge
    slot j of segment (t,q) at block boff[t][q]+j//128, partition j%128 —
    exactly dma_gather's output layout."""
    sb = np.stack([(c["cnt_tq"] + P - 1) // P for c in branch_cores]).max(axis=0)
    ngroups = _ceil(T, TG)
    boff = np.zeros((T, NQ), int)
    goff = np.zeros(ngroups + 1, int)
    gb = np.zeros((ngroups, NQ), int)
    cum = 0
    for g in range(ngroups):
        goff[g] = cum
        for q in range(NQ):
            for t in range(g * TG, min((g + 1) * TG, T)):
                boff[t][q] = cum
                cum += sb[t][q]
                gb[g][q] += sb[t][q]
    goff[ngroups] = cum
    mb = sb.sum(axis=1)
    Mbar = max(1, int(cum))
    assert not (K_REG and TG != 1), "K_REG needs call-trailing pads (TG==1)"
    calls = [(t, q) for t in range(T) for q in range(NQ) if sb[t][q] > 0]
    out = []
    for c in branch_cores:
        F = np.full(Mbar * P, -1 if K_REG else 0, np.int16)
        DSTL = np.full((P, Mbar), -1.0, BF)
        es, ed, tl, q, cnt_tq = (c["es"], c["ed"], c["tl"], c["q"], c["cnt_tq"])
        if len(ed):
            segid = tl * NQ + q
            starts = np.concatenate([[0], np.cumsum(cnt_tq.ravel())])
            within = np.arange(len(ed)) - starts[segid]
            flat = boff.ravel()[segid] * P + within
            F[flat] = (es & (CHK - 1)).astype(np.int16)
            DSTL[flat & 127, flat >> 7] = (ed - (tl << 7)).astype(np.float32)
        IDX16 = np.ascontiguousarray(np.tile(F.reshape(-1, 16).T, (8, 1)))
        CNT = np.array([[int(cnt_tq[t][q]) for (t, q) in calls]], np.int32)
        out.append({"IDX16": IDX16, "DSTL": DSTL, "CNT": CNT})
    return (sb.astype(int), boff.astype(int), mb.astype(int).tolist(),
            goff.astype(int), gb.astype(int), Mbar, len(calls), out)


def _part_major(vec, TB, fill):
    v = np.full(TB * P, fill, vec.dtype)
    v[: len(vec)] = vec
    return np.ascontiguousarray(v.reshape(TB, P).T)


def preprocess(x, x_da, edge_index, batch, rootindex):
    import scipy.sparse as sp
    N = x.shape[0]
    B = rootindex.shape[0]
    x0 = np.concatenate([x, x_da], axis=1).astype(np.float32)
    assert x0.shape[1] == P
    TBL = _ceil(N, P)
    x0T = np.zeros((P, TBL * P), BF)
    x0T[:, :N] = x0.T.astype(BF)
    batch = batch.astype(np.int64)
    meta = _shard_meta(batch, B, N)
    T, NLOC, G_LOC = meta["T"], meta["NLOC"], meta["G_LOC"]
    B_PAD = _ceil(B, P) * P

    src_g = edge_index[0].astype(np.int64)
    dst_g = edge_index[1].astype(np.int64)

    NQ = _ceil(TBL * P, CHK)
    branches = {}
    for name, (s, d) in {"td": (src_g, dst_g), "bu": (dst_g, src_g)}.items():
        deg = (np.bincount(d, minlength=N) + 1.0).astype(np.float64)
        dinv = (1.0 / np.sqrt(deg)).astype(np.float32)
        sqdeg = np.sqrt(deg).astype(np.float32)
        cores = []
        for c in range(N_CORES):
            es, ed, tl, q, cnt_tq = _edges_for_core(
                s, d, meta["n0"][c], NLOC, N, T, NQ)
            cores.append({"es": es, "ed": ed, "tl": tl, "q": q,
                          "cnt_tq": cnt_tq})
        sb, boff, mb, goff, gb, Mbar, ncalls, packed = _pack_edges(cores, T, NQ)
        bd = {"dinv": dinv, "sb": sb, "boff": boff, "mbar": mb,
              "goff": goff, "gb": gb, "Mbar": Mbar, "ncalls": ncalls,
              "packed": packed,
              "dinv_part": _part_major(dinv, TBL, np.float32(1.0))}
        loc_dinv, loc_sqdeg, loc_C = [], [], []
        for c in range(N_CORES):
            rows = meta["n0"][c] + np.arange(NLOC)
            valid = rows < N
            rr = np.minimum(rows, N - 1)
            dv = np.where(valid, dinv[rr], 1.0).astype(np.float32)
            loc_dinv.append(np.ascontiguousarray(dv.reshape(T, P).T))
            sq = np.where(valid, sqdeg[rr], 0.0).astype(BF)
            loc_sqdeg.append(np.ascontiguousarray(sq.reshape(1, NLOC)))
            # C[g, dloc] = sum_{e: s->d, d local} dinv[s] * [batch[s] == g]
            # (+ self-loop dinv[d] at batch[d]) — layer-2 root-extend operand.
            ec = cores[c]
            gsrc = batch[ec["es"]]
            w = dinv[ec["es"]]
            Cm = sp.coo_matrix((w, (gsrc, ec["ed"])), shape=(B_PAD, NLOC)).toarray()
            loc_C.append(np.ascontiguousarray(Cm.astype(BF)))
        bd["dinv_loc"] = loc_dinv
        bd["sqdeg_loc"] = loc_sqdeg
        bd["C"] = loc_C
        branches[name] = bd

    GL, cinv = [], []
    for c in range(N_CORES):
        rows = meta["n0"][c] + np.arange(NLOC)
        valid = rows < N
        rr = np.minimum(rows, N - 1)
        gl = np.where(valid, batch[rr] - meta["g0"][c], -1).astype(np.float32)
        gl = np.where(gl < G_LOC, gl, -1.0).astype(np.float32)
        GL.append(np.ascontiguousarray(gl.reshape(T, P).T.astype(BF)))
        cnts = np.ones(P, np.float32)
        ns = meta["node_start"]
        for j in range(G_LOC):
            g = meta["g0"][c] + j
            if g < B:
                cc = float(ns[g + 1] - ns[g])
                cnts[j] = cc if cc > 0 else 1.0
        cinv.append((1.0 / cnts).reshape(P, 1).astype(np.float32))

    rootx0T = np.zeros((P, B_PAD), BF)
    rootx0T[:, :B] = np.maximum(x0[rootindex.astype(np.int64)], 0.0).T.astype(BF)

    iota = np.broadcast_to(np.arange(P, dtype=np.float32), (P, P)).astype(BF).copy()
    ident = np.eye(P, dtype=np.float32)

    return {"N": N, "B": B, "TBL": TBL, "B_PAD": B_PAD, "NQ": NQ,
            "meta": meta, "x0": x0, "x0T": x0T, "branches": branches,
            "GL": GL, "cinv": cinv, "rootx0T": rootx0T,
            "iota": iota, "ident": ident}


# ----------------------------------------------------------------------------
# device program builders
# ----------------------------------------------------------------------------

import os
N_QUEUES = int(os.environ.get("K_QUEUES", "4"))
K_SP = os.environ.get("K_SP", "1") == "1"   # single_packet for gathers
K_ROT = os.environ.get("K_ROT", "0") == "1"  # rotate gathers across queues (UNSAFE: sem-lane/queue ordering)
_qctr = [0]


def _next_q():
    if not K_ROT:
        return 0
    q = _qctr[0] % N_QUEUES
    _qctr[0] += 1
    return q


# SWDGE descriptor-ring carveout (bytes/partition). 48KB lets a gather's
# descriptor generation run ahead of the previous gather's SDMA drain
# (default 16KB holds ~one call's descriptors → gen/drain serialize).
K_SCRATCH = int(os.environ.get("K_SCRATCH", "49152"))


def _new_nc():
    return bacc.Bacc("TRN2", target_bir_lowering=False, debug=False,
                     num_devices=N_CORES, num_swdge_queues=N_QUEUES,
                     dynamic_dma_scratch_size=K_SCRATCH)


def _load(nc, pool, dram_ap, shape, dtype, tag, bufs=1):
    t = pool.tile(list(shape), dtype, tag=tag, bufs=bufs)
    nc.sync.dma_start(out=t[:], in_=dram_ap)
    return t


def _aggregate(nc, pool, psum, table, IDX_sb, DSTL_sb, bmeta, TROWS,
               iota_sb, T, pre, npre, consume, cnt_sb=None, regs=None):
    """Group-merged segment-sum: per group of TG dst tiles, one dma_gather
    per 32k-row table chunk fetches all the group's messages, one vector op
    builds all its one-hot blocks, then per tile: PSUM_t = pre-matmuls +
    sum_e A01 . msg (self-loops are real edges) and consume(t, ps) finishes
    (scale/relu/pool/write)."""
    sb, boff, mbar = bmeta["sb"], bmeta["boff"], bmeta["mbar"]
    goff, gb = bmeta["goff"], bmeta["gb"]
    NQ = sb.shape[1]
    ngroups = len(gb)
    gsmax = max(1, int((goff[1:] - goff[:-1]).max()))
    call_i = 0
    for g in range(ngroups):
        gs = int(goff[g + 1] - goff[g])
        if gs == 0:
            continue  # window tail beyond N: no nodes, nothing to write
        msg = pool.tile([P, gsmax * P], BF16, tag="msg", bufs=3 if TG == 1 else 2)
        col = 0
        for q in range(NQ):
            nb = int(gb[g][q])
            if nb == 0:
                continue
            base = q * CHK
            rows = min(CHK, TROWS - base)
            b0 = int(goff[g]) + col
            if cnt_sb is not None:
                reg = regs[call_i % len(regs)]
                nc.gpsimd.reg_load(reg, cnt_sb[0:1, call_i: call_i + 1])
                nreg = reg
            else:
                nreg = nb * P
            nc.gpsimd.dma_gather(
                out_ap=msg[:, col * P: (col + nb) * P]
                .rearrange("p (b f) -> p b f", f=P),
                in_ap=table[base: base + rows, :],
                idxs_ap=IDX_sb[:, b0 * 8: (b0 + nb) * 8],
                num_idxs=nb * P, num_idxs_reg=nreg, elem_size=P,
                queue_num=_next_q(), single_packet=K_SP)
            call_i += 1
            col += nb
        a01 = pool.tile([P, gsmax * P], BF16, tag="a01", bufs=3 if TG == 1 else 2)
        nc.vector.tensor_tensor(
            out=a01[:, : gs * P].rearrange("p (k f) -> p k f", f=P),
            in0=DSTL_sb[:, goff[g]: goff[g] + gs].to_broadcast([P, gs, P]),
            in1=iota_sb[:].unsqueeze(1).broadcast_to([P, gs, P]),
            op=mybir.AluOpType.is_equal,
        )
        for t in range(g * TG, min((g + 1) * TG, T)):
            if mbar[t] == 0:
                continue
            ps = psum.tile([P, P], F32, tag="ps")
            pre(t, ps)  # npre matmuls, first with start=True
            nk, klast = 0, mbar[t] - 1
            for q in range(NQ):
                for j in range(int(sb[t][q])):
                    k = int(boff[t][q]) - int(goff[g]) + j
                    nc.tensor.matmul(ps[:], lhsT=a01[:, k * P: (k + 1) * P],
                                     rhs=msg[:, k * P: (k + 1) * P],
                                     start=(npre == 0 and nk == 0),
                                     stop=(nk == klast))
                    nk += 1
            consume(t, ps)


def build_l1(pp, reps=1):
    _qctr[0] = 0
    TBL, T = pp["TBL"], pp["meta"]["T"]
    NLOC = pp["meta"]["NLOC"]
    br = pp["branches"]
    nc = _new_nc()
    x0T = nc.dram_tensor("x0T", [P, TBL * P], BF16, kind="ExternalInput")
    iota = nc.dram_tensor("iota", [P, P], BF16, kind="ExternalInput")
    w12 = nc.dram_tensor("w12", [P, 2 * P], BF16, kind="ExternalInput")
    ins = {}
    for b in ("td", "bu"):
        M = br[b]["Mbar"]
        ins[b] = {
            "b1": nc.dram_tensor(f"b1{b}", [1, P], BF16, kind="ExternalInput"),
            "dinv_part": nc.dram_tensor(f"dinvp{b}", [P, TBL], F32, kind="ExternalInput"),
            "dinv_loc": nc.dram_tensor(f"dinvl{b}", [P, T], F32, kind="ExternalInput"),
            "sqdeg": nc.dram_tensor(f"sqdeg{b}", [1, NLOC], BF16, kind="ExternalInput"),
            "IDX": nc.dram_tensor(f"IDX{b}", [P, M * 8], I16, kind="ExternalInput"),
            "DSTL": nc.dram_tensor(f"DSTL{b}", [P, M], BF16, kind="ExternalInput"),
            "table": nc.dram_tensor(f"table{b}", [TBL * P, P], BF16, kind="Internal"),
            "x2": nc.dram_tensor(f"x2{b}", [T * P, P], F32, kind="ExternalOutput"),
        }
        if K_REG:
            ins[b]["CNT"] = nc.dram_tensor(
                f"CNT{b}", [1, br[b]["ncalls"]], mybir.dt.int32,
                kind="ExternalInput")

    with tile.TileContext(nc) as tc:
        with (
            tc.tile_pool(name="sbuf", bufs=2) as pool,
            tc.tile_pool(name="cst", bufs=1) as cst,
            tc.tile_pool(name="psum", bufs=4, space="PSUM") as psum,
        ):
            iota_sb = _load(nc, cst, iota[:], (P, P), BF16, "iota")
            w12_sb = _load(nc, cst, w12[:], (P, 2 * P), BF16, "w12")
            dinvp_sb = {b: _load(nc, cst, ins[b]["dinv_part"][:], (P, TBL), F32,
                                 f"dinvp{b}") for b in ("td", "bu")}

            import contextlib
            loop_ctx = tc.For_i(0, reps, 1) if reps > 1 else contextlib.nullcontext()
            with loop_ctx:
                _build_l1_body(nc, x0T, pool, cst, psum, pp, ins, iota_sb,
                               w12_sb, dinvp_sb)
    nc.compile()
    return nc


def _build_l1_body(nc, x0T, pool, cst, psum, pp, ins, iota_sb, w12_sb,
                   dinvp_sb):
    TBL, T = pp["TBL"], pp["meta"]["T"]
    br = pp["branches"]
    # ---- tables: ht_b = dinv_b * (x0 @ W1_b), node-major bf16 in DRAM ----
    CH = 8
    for c0 in range(0, TBL, CH):
        nb = min(CH, TBL - c0)
        xt = pool.tile([P, CH * P], BF16, tag="xt", bufs=2)
        nc.sync.dma_start(out=xt[:, : nb * P],
                          in_=x0T[:, c0 * P: (c0 + nb) * P])
        st = {b: pool.tile([P, CH * P], BF16, tag=f"st{b}", bufs=2,
                           name=f"st{b}")
              for b in ("td", "bu")}
        for j in range(nb):
            blk = c0 + j
            psx = psum.tile([P, 2 * P], F32, tag="psx")
            nc.tensor.matmul(psx[:], lhsT=xt[:, j * P: (j + 1) * P],
                             rhs=w12_sb[:], start=True, stop=True)
            for bi, b in enumerate(("td", "bu")):
                nc.scalar.activation(
                    out=st[b][:, j * P: (j + 1) * P],
                    in_=psx[:, bi * P: (bi + 1) * P],
                    func=mybir.ActivationFunctionType.Copy,
                    scale=dinvp_sb[b][:, blk: blk + 1])
        for b in ("td", "bu"):
            nc.sync.dma_start(
                out=ins[b]["table"][c0 * P: (c0 + nb) * P, :]
                .rearrange("(j p) f -> p j f", p=P),
                in_=st[b][:, : nb * P].rearrange("p (j f) -> p j f", f=P))

    # ---- aggregation per branch ----
    regs = ([nc.gpsimd.alloc_register(f"cnt{i}") for i in range(4)]
            if K_REG else None)
    for b in ("td", "bu"):
        ib = ins[b]
        M = br[b]["Mbar"]
        IDX_sb = _load(nc, pool, ib["IDX"][:], (P, M * 8), I16, "idx")
        DSTL_sb = _load(nc, pool, ib["DSTL"][:], (P, M), BF16, "dstl")
        dinvl_sb = _load(nc, cst, ib["dinv_loc"][:], (P, T), F32, f"dinvl{b}")
        sqdeg_sb = _load(nc, cst, ib["sqdeg"][:], (1, T * P), BF16, f"sqdeg{b}")
        b1_sb = _load(nc, cst, ib["b1"][:], (1, P), BF16, f"b1{b}")
        cnt_sb = (_load(nc, pool, ib["CNT"][:], (1, br[b]["ncalls"]),
                        mybir.dt.int32, "cnt") if K_REG else None)

        def pre(t, ps, sqdeg_sb=sqdeg_sb, b1_sb=b1_sb):
            nc.tensor.matmul(ps[:], lhsT=sqdeg_sb[0:1, t * P: (t + 1) * P],
                             rhs=b1_sb[0:1, :], start=True, stop=False)

        def consume(t, ps, ib=ib, dinvl_sb=dinvl_sb):
            xo = pool.tile([P, P], F32, tag="xo", bufs=3)
            nc.scalar.activation(out=xo[:], in_=ps[:],
                                 func=mybir.ActivationFunctionType.Copy,
                                 scale=dinvl_sb[:, t: t + 1])
            nc.sync.dma_start(out=ib["x2"][t * P: (t + 1) * P, :], in_=xo[:])

        _aggregate(nc, pool, psum, ib["table"], IDX_sb, DSTL_sb,
                   br[b], TBL * P, iota_sb, T, pre, 1, consume,
                   cnt_sb=cnt_sb, regs=regs)


def build_l2(pp, reps=1):
    _qctr[0] = 0
    TBL, T = pp["TBL"], pp["meta"]["T"]
    G_LOC, B_PAD = pp["meta"]["G_LOC"], pp["B_PAD"]
    NLOC = pp["meta"]["NLOC"]
    br = pp["branches"]
    nc = _new_nc()
    iota = nc.dram_tensor("iota", [P, P], BF16, kind="ExternalInput")
    ident = nc.dram_tensor("ident", [P, P], F32, kind="ExternalInput")
    GL = nc.dram_tensor("GL", [P, T], BF16, kind="ExternalInput")
    cinv = nc.dram_tensor("cinv", [P, 1], F32, kind="ExternalInput")
    rootx0T = nc.dram_tensor("rootx0T", [P, B_PAD], BF16, kind="ExternalInput")
    ones_row = nc.dram_tensor("ones_row", [1, P], F32, kind="ExternalInput")
    mlp_w1 = nc.dram_tensor("mlp_w1", [4 * P, 2 * P], F32, kind="ExternalInput")
    mlp_b1 = nc.dram_tensor("mlp_b1", [1, 2 * P], F32, kind="ExternalInput")
    mlp_w2 = nc.dram_tensor("mlp_w2", [2 * P, 2], F32, kind="ExternalInput")
    mlp_b2 = nc.dram_tensor("mlp_b2", [1, 2], F32, kind="ExternalInput")
    out = nc.dram_tensor("out", [P, 2], F32, kind="ExternalOutput")
    NCH = B_PAD // P  # 4 root/graph chunks
    ins = {}
    for b in ("td", "bu"):
        M = br[b]["Mbar"]
        ins[b] = {
            "x2rT": nc.dram_tensor(f"x2rT{b}", [P, TBL * P], BF16, kind="ExternalInput"),
            "w2a": nc.dram_tensor(f"w2a{b}", [P, P], BF16, kind="ExternalInput"),
            "w2b": nc.dram_tensor(f"w2b{b}", [P, P], BF16, kind="ExternalInput"),
            "b2": nc.dram_tensor(f"b2{b}", [1, P], BF16, kind="ExternalInput"),
            "dinv_part": nc.dram_tensor(f"dinvp{b}", [P, TBL], F32, kind="ExternalInput"),
            "dinv_loc": nc.dram_tensor(f"dinvl{b}", [P, T], F32, kind="ExternalInput"),
            "sqdeg": nc.dram_tensor(f"sqdeg{b}", [1, NLOC], BF16, kind="ExternalInput"),
            "C": nc.dram_tensor(f"C{b}", [B_PAD, NLOC], BF16, kind="ExternalInput"),
            "IDX": nc.dram_tensor(f"IDX{b}", [P, M * 8], I16, kind="ExternalInput"),
            "DSTL": nc.dram_tensor(f"DSTL{b}", [P, M], BF16, kind="ExternalInput"),
            "rootx2T": nc.dram_tensor(f"rx2T{b}", [P, P], F32, kind="ExternalInput"),
            "table": nc.dram_tensor(f"table{b}", [TBL * P, P], BF16, kind="Internal"),
        }
        if K_REG:
            ins[b]["CNT"] = nc.dram_tensor(
                f"CNT{b}", [1, br[b]["ncalls"]], mybir.dt.int32,
                kind="ExternalInput")

    with tile.TileContext(nc) as tc:
        with (
            tc.tile_pool(name="sbuf", bufs=2) as pool,
            tc.tile_pool(name="cst", bufs=1) as cst,
            tc.tile_pool(name="psum", bufs=4, space="PSUM") as psum,
            tc.tile_pool(name="pps", bufs=1, space="PSUM") as pool_ps,
        ):
            iota_sb = _load(nc, cst, iota[:], (P, P), BF16, "iota")
            ident_sb = _load(nc, cst, ident[:], (P, P), F32, "ident")
            ones_sb = _load(nc, cst, ones_row[:], (1, P), F32, "ones")
            GL_sb = _load(nc, cst, GL[:], (P, T), BF16, "GL")
            cinv_sb = _load(nc, cst, cinv[:], (P, 1), F32, "cinv")
            rx0_sb = _load(nc, cst, rootx0T[:], (P, B_PAD), BF16, "rx0")
            w2a_sb = {b: _load(nc, cst, ins[b]["w2a"][:], (P, P), BF16, f"w2a{b}")
                      for b in ("td", "bu")}
            w2b_sb = {b: _load(nc, cst, ins[b]["w2b"][:], (P, P), BF16, f"w2b{b}")
                      for b in ("td", "bu")}
            dinvp_sb = {b: _load(nc, cst, ins[b]["dinv_part"][:], (P, TBL), F32,
                                 f"dinvp{b}") for b in ("td", "bu")}

            import contextlib
            loop_ctx = tc.For_i(0, reps, 1) if reps > 1 else contextlib.nullcontext()
            with loop_ctx:
                _build_l2_body(
                    nc, pool, cst, psum, pool_ps, pp, ins, mlp_w1,
                    mlp_b1, mlp_w2, mlp_b2, out, iota_sb, ident_sb, ones_sb,
                    GL_sb, cinv_sb, rx0_sb, w2a_sb, w2b_sb, dinvp_sb, NCH)
    nc.compile()
    return nc


def _build_l2_body(nc, pool, cst, psum, pool_ps, pp, ins, mlp_w1,
                   mlp_b1, mlp_w2, mlp_b2, out, iota_sb, ident_sb, ones_sb,
                   GL_sb, cinv_sb, rx0_sb, w2a_sb, w2b_sb, dinvp_sb, NCH):
    TBL, T = pp["TBL"], pp["meta"]["T"]
    G_LOC, B_PAD = pp["meta"]["G_LOC"], pp["B_PAD"]
    br = pp["branches"]
    # ---- R_b = relu(x0[roots]) @ W2b_b, SBUF-resident [128g x 128f] x NCH ----
    R_sb = {}
    for b in ("td", "bu"):
        R_sb[b] = cst.tile([P, NCH * P], BF16, tag=f"R{b}", name=f"R{b}")
        for j in range(NCH):
            psr = psum.tile([P, P], F32, tag="ps")
            nc.tensor.matmul(psr[:], lhsT=rx0_sb[:, j * P: (j + 1) * P],
                             rhs=w2b_sb[b][:], start=True, stop=True)
            nc.scalar.activation(out=R_sb[b][:, j * P: (j + 1) * P], in_=psr[:],
                                 func=mybir.ActivationFunctionType.Copy)

    # ---- ht2 tables: dinv_b * (relu(x2_b) @ W2a_b), bf16 in DRAM ----
    CH = 8
    for b in ("td", "bu"):
        ib = ins[b]
        for c0 in range(0, TBL, CH):
            nb = min(CH, TBL - c0)
            xt = pool.tile([P, CH * P], BF16, tag="xt", bufs=2)
            nc.sync.dma_start(out=xt[:, : nb * P],
                              in_=ib["x2rT"][:, c0 * P: (c0 + nb) * P])
            st = pool.tile([P, CH * P], BF16, tag="st", bufs=2)
            for j in range(nb):
                blk = c0 + j
                psx = psum.tile([P, P], F32, tag="ps")
                nc.tensor.matmul(psx[:], lhsT=xt[:, j * P: (j + 1) * P],
                                 rhs=w2a_sb[b][:], start=True, stop=True)
                nc.scalar.activation(
                    out=st[:, j * P: (j + 1) * P], in_=psx[:],
                    func=mybir.ActivationFunctionType.Copy,
                    scale=dinvp_sb[b][:, blk: blk + 1])
            nc.sync.dma_start(
                out=ib["table"][c0 * P: (c0 + nb) * P, :]
                .rearrange("(j p) f -> p j f", p=P),
                in_=st[:, : nb * P].rearrange("p (j f) -> p j f", f=P))

    # ---- aggregation + relu + pooling per branch ----
    pooled = {}
    regs = ([nc.gpsimd.alloc_register(f"cnt{i}") for i in range(4)]
            if K_REG else None)
    for b in ("td", "bu"):
        ib = ins[b]
        M = br[b]["Mbar"]
        IDX_sb = _load(nc, pool, ib["IDX"][:], (P, M * 8), I16, "idx")
        DSTL_sb = _load(nc, pool, ib["DSTL"][:], (P, M), BF16, "dstl")
        cnt_sb = (_load(nc, pool, ib["CNT"][:], (1, br[b]["ncalls"]),
                        mybir.dt.int32, "cnt") if K_REG else None)
        dinvl_sb = _load(nc, cst, ib["dinv_loc"][:], (P, T), F32, f"dinvl{b}")
        sqdeg_sb = _load(nc, cst, ib["sqdeg"][:], (1, T * P), BF16, f"sqdeg{b}")
        b2_sb = _load(nc, cst, ib["b2"][:], (1, P), BF16, f"b2{b}")
        ps_pool = pool_ps.tile([G_LOC, P], F32, tag=f"pool{b}")
        t_last = max(t for t in range(T) if br[b]["mbar"][t] > 0)

        def pre(t, ps, ib=ib, sqdeg_sb=sqdeg_sb, b2_sb=b2_sb, Rb=R_sb[b]):
            nc.tensor.matmul(ps[:], lhsT=sqdeg_sb[0:1, t * P: (t + 1) * P],
                             rhs=b2_sb[0:1, :], start=True, stop=False)
            ct = pool.tile([P, NCH * P], BF16, tag="ct", bufs=3)
            nc.sync.dma_start(
                out=ct[:].rearrange("g (c d) -> g c d", d=P),
                in_=ib["C"][:, t * P: (t + 1) * P]
                .rearrange("(c g) d -> g c d", g=P))
            for ci in range(NCH):
                nc.tensor.matmul(ps[:], lhsT=ct[:, ci * P: (ci + 1) * P],
                                 rhs=Rb[:, ci * P: (ci + 1) * P],
                                 start=False, stop=False)

        def consume(t, ps, dinvl_sb=dinvl_sb, ps_pool=ps_pool, t_last=t_last):
            h2 = pool.tile([P, P], BF16, tag="h2", bufs=3)
            nc.scalar.activation(out=h2[:], in_=ps[:],
                                 func=mybir.ActivationFunctionType.Relu,
                                 scale=dinvl_sb[:, t: t + 1])
            oh = pool.tile([P, G_LOC], BF16, tag="oh", bufs=3)
            nc.vector.tensor_tensor(
                out=oh[:], in0=GL_sb[:, t: t + 1].to_broadcast([P, G_LOC]),
                in1=iota_sb[:, :G_LOC], op=mybir.AluOpType.is_equal)
            nc.tensor.matmul(ps_pool[:], lhsT=oh[:], rhs=h2[:],
                             start=(t == 0), stop=(t == t_last))

        _aggregate(nc, pool, psum, ib["table"], IDX_sb, DSTL_sb,
                   br[b], TBL * P, iota_sb, T, pre, 1 + NCH, consume,
                   cnt_sb=cnt_sb, regs=regs)

        meanS = pool.tile([G_LOC, P], F32, tag=f"mean{b}", bufs=1)
        nc.vector.tensor_scalar(
            out=meanS[:], in0=ps_pool[:], scalar1=cinv_sb[:G_LOC, :],
            scalar2=None, op0=mybir.AluOpType.mult)
        pst = psum.tile([P, G_LOC], F32, tag="ps")
        nc.tensor.transpose(out=pst[:], in_=meanS[:],
                            identity=ident_sb[:G_LOC, :G_LOC])
        meanT = pool.tile([P, G_LOC], F32, tag=f"meanT{b}", bufs=1)
        nc.vector.tensor_copy(out=meanT[:], in_=pst[:])
        pooled[b] = meanT

    # ---- final MLP over this core's G_LOC graph slots ----
    rx2_sb = {b: _load(nc, cst, ins[b]["rootx2T"][:, :P], (P, P), F32,
                       f"rx2{b}") for b in ("td", "bu")}
    w1c_sb = cst.tile([P, 4 * 2 * P], F32, tag="mw1", name="mw1")
    nc.sync.dma_start(
        out=w1c_sb[:].rearrange("p (c o) -> p c o", c=4),
        in_=mlp_w1[:].rearrange("(c p) o -> p c o", p=P))
    b1m_sb = _load(nc, cst, mlp_b1[:], (1, 2 * P), F32, "mb1")
    w2c_sb = cst.tile([P, 2 * 2], F32, tag="mw2", name="mw2")
    nc.sync.dma_start(
        out=w2c_sb[:].rearrange("p (c o) -> p c o", c=2),
        in_=mlp_w2[:].rearrange("(c p) o -> p c o", p=P))
    b2m_sb = _load(nc, cst, mlp_b2[:], (1, 2), F32, "mb2")

    ps1 = psum.tile([G_LOC, 2 * P], F32, tag="ps")
    chunks = [pooled["bu"][:, :G_LOC], rx2_sb["bu"][:, :G_LOC],
              pooled["td"][:, :G_LOC], rx2_sb["td"][:, :G_LOC]]
    for ci, lhsT in enumerate(chunks):
        nc.tensor.matmul(ps1[:], lhsT=lhsT,
                         rhs=w1c_sb[:, ci * 2 * P: (ci + 1) * 2 * P],
                         start=(ci == 0), stop=False)
    nc.tensor.matmul(ps1[:], lhsT=ones_sb[0:1, :G_LOC], rhs=b1m_sb[0:1, :],
                     start=False, stop=True)
    h1 = pool.tile([G_LOC, 2 * P], F32, tag="mlph", bufs=1)
    nc.scalar.activation(out=h1[:], in_=ps1[:],
                         func=mybir.ActivationFunctionType.Relu)
    hT = []
    for ci in range(2):
        pst2 = psum.tile([P, G_LOC], F32, tag="ps")
        nc.tensor.transpose(out=pst2[:], in_=h1[:, ci * P: (ci + 1) * P],
                            identity=ident_sb[:G_LOC, :G_LOC])
        ht_sb = pool.tile([P, G_LOC], F32, tag=f"hT{ci}", bufs=1)
        nc.vector.tensor_copy(out=ht_sb[:], in_=pst2[:])
        hT.append(ht_sb)
    ps2 = psum.tile([G_LOC, 2], F32, tag="ps")
    for ci in range(2):
        nc.tensor.matmul(ps2[:], lhsT=hT[ci][:, :G_LOC],
                         rhs=w2c_sb[:, ci * 2: (ci + 1) * 2],
                         start=(ci == 0), stop=False)
    nc.tensor.matmul(ps2[:], lhsT=ones_sb[0:1, :G_LOC], rhs=b2m_sb[0:1, :],
                     start=False, stop=True)
    oo = pool.tile([G_LOC, 2], F32, tag="oo", bufs=1)
    nc.vector.tensor_copy(out=oo[:], in_=ps2[:])
    nc.sync.dma_start(out=out[:G_LOC, :], in_=oo[:])


# ----------------------------------------------------------------------------
# in_map assembly + kernel entry
# ----------------------------------------------------------------------------

def l1_in_maps(pp, w):
    br = pp["branches"]
    w12 = np.concatenate([w["td_w1"], w["bu_w1"]], axis=1).astype(BF)
    maps = []
    for c in range(N_CORES):
        m = {"x0T": pp["x0T"], "iota": pp["iota"],
             "w12": np.ascontiguousarray(w12)}
        for b in ("td", "bu"):
            bb = br[b]
            m[f"b1{b}"] = w[f"{b}_b1"].astype(BF).reshape(1, P)
            m[f"dinvp{b}"] = bb["dinv_part"]
            m[f"dinvl{b}"] = bb["dinv_loc"][c]
            m[f"sqdeg{b}"] = bb["sqdeg_loc"][c]
            m[f"IDX{b}"] = bb["packed"][c]["IDX16"]
            m[f"DSTL{b}"] = bb["packed"][c]["DSTL"]
            if K_REG:
                m[f"CNT{b}"] = bb["packed"][c]["CNT"]
        maps.append(m)
    return maps


def l2_in_maps(pp, w, x2rT, rootx2T):
    br = pp["branches"]
    ones = np.ones((1, P), np.float32)
    maps = []
    for c in range(N_CORES):
        m = {"iota": pp["iota"], "ident": pp["ident"],
             "GL": pp["GL"][c],
             "cinv": pp["cinv"][c], "rootx0T": pp["rootx0T"], "ones_row": ones,
             "mlp_w1": w["mlp_w1"].astype(np.float32),
             "mlp_b1": w["mlp_b1"].astype(np.float32).reshape(1, -1),
             "mlp_w2": w["mlp_w2"].astype(np.float32),
             "mlp_b2": w["mlp_b2"].astype(np.float32).reshape(1, -1)}
        for b in ("td", "bu"):
            bb = br[b]
            m[f"x2rT{b}"] = x2rT[b]
            m[f"w2a{b}"] = np.ascontiguousarray(w[f"{b}_w2"][:P].astype(BF))
            m[f"w2b{b}"] = np.ascontiguousarray(w[f"{b}_w2"][P:].astype(BF))
            m[f"b2{b}"] = w[f"{b}_b2"].astype(BF).reshape(1, P)
            m[f"dinvp{b}"] = bb["dinv_part"]
            m[f"dinvl{b}"] = bb["dinv_loc"][c]
            m[f"sqdeg{b}"] = bb["sqdeg_loc"][c]
            m[f"C{b}"] = bb["C"][c]
            m[f"IDX{b}"] = bb["packed"][c]["IDX16"]
            m[f"DSTL{b}"] = bb["packed"][c]["DSTL"]
            if K_REG:
                m[f"CNT{b}"] = bb["packed"][c]["CNT"]
            m[f"rx2T{b}"] = rootx2T[b][c]
        maps.append(m)
    return maps


def assemble_x2(pp, results, b):
    N, meta = pp["N"], pp["meta"]
    ns, g0 = meta["node_start"], meta["g0"]
    x2 = np.zeros((N, P), np.float32)
    for c in range(N_CORES):
        lo, hi = int(ns[g0[c]]), int(ns[g0[c + 1]])
        x2[lo:hi] = results[c][f"x2{b}"][: hi - lo]
    return x2


def make_x2rT(pp, x2):
    """[128, TBL*128] bf16 relu(x2) transposed — layer-2 table-build input."""
    N, TBL = pp["N"], pp["TBL"]
    xt = np.zeros((P, TBL * P), BF)
    xt[:, :N] = np.maximum(x2, 0.0).T.astype(BF)
    return xt


def make_rootx2T(pp, x2, rootindex):
    """Per-core [128, 128] (padded from G_LOC) transposed root features."""
    meta = pp["meta"]
    G_LOC = meta["G_LOC"]
    root = np.asarray(rootindex).astype(np.int64)
    percore = []
    for c in range(N_CORES):
        gi = np.minimum(meta["g0"][c] + np.arange(G_LOC), pp["B"] - 1)
        rt = np.zeros((P, P), np.float32)
        rt[:, :G_LOC] = x2[root[gi]].T
        percore.append(rt)
    return percore


def _run(nc, in_maps):
    return run_bass_kernel_spmd(nc, in_maps, core_ids=list(range(N_CORES))).results


def kernel(x, x_da, edge_index, batch, rootindex,
           td_w1, td_b1, td_w2, td_b2,
           bu_w1, bu_b1, bu_w2, bu_b2,
           mlp_w1, mlp_b1, mlp_w2, mlp_b2):
    w = {"td_w1": td_w1, "td_b1": td_b1, "td_w2": td_w2, "td_b2": td_b2,
         "bu_w1": bu_w1, "bu_b1": bu_b1, "bu_w2": bu_w2, "bu_b2": bu_b2,
         "mlp_w1": mlp_w1, "mlp_b1": mlp_b1, "mlp_w2": mlp_w2, "mlp_b2": mlp_b2}
    w = {k: np.asarray(v) for k, v in w.items()}
    pp = preprocess(np.asarray(x), np.asarray(x_da), np.asarray(edge_index),
                    np.asarray(batch), np.asarray(rootindex))

    nc1 = build_l1(pp)
    res1 = _run(nc1, l1_in_maps(pp, w))

    x2rT, rootx2T = {}, {}
    for b in ("td", "bu"):
        x2 = assemble_x2(pp, res1, b)
        x2rT[b] = make_x2rT(pp, x2)
        rootx2T[b] = make_rootx2T(pp, x2, rootindex)

    nc2 = build_l2(pp)
    res2 = _run(nc2, l2_in_maps(pp, w, x2rT, rootx2T))

    B = pp["B"]
    meta = pp["meta"]
    out = np.zeros((B, 2), np.float32)
    for c in range(N_CORES):
        g0, g1 = meta["g0"][c], meta["g0"][c + 1]
        out[g0:g1] = res2[c]["out"][: g1 - g0]
    return out
